# revision 19
# baseline (speedup 1.0000x reference)
"""GQA attention with QK-norm for Trainium2, sharded over 8 NeuronCores.

Problem: B=2, T=2048, D=2048, H=16 query heads, KVH=4 kv heads, dk=128.
    Q = q @ Wq.T ; K = k @ Wk.T ; V = v @ Wv.T  (per batch)
    Q = g * l2norm(Q, per head) ; K = l2norm(K, per head)
    out = softmax(causal(Q K^T / sqrt(dk))) V @ Wo.T

Sharding: core c = 4*b + gi handles batch b and kv-head group gi
(4 query heads + 1 kv head). Each core computes a row-shard of the
output projection (O^T partial over its 512 head-dims); the host sums
the 4 partials per batch. No device collectives.

Schedule (v2): PE warm-up runs from t~0 on a Pool-memset scratch
tile.  K-proj runs interleaved with Q-proj head 0 (K in psacc, Q0 in
the psy/pso PSUM slots) with fp8(e4m3)+DoubleRow matmuls (the x32
weight scale cancels in the post-projection l2norm); V-proj (bf16)
interleaves with Q-proj h1 the same way; norm scale chains are
DVE-only (reciprocal + Newton rsqrt, no ACT-table thrash), staged
through DRAM with single strided-partition DMAs.  V transposes stay
on the PE (XBAR DMA-transpose reads are not hazard-tracked by Tile
and race the vt_stage drain).  stage1 for q-blocks 0/1 is emitted
per-head
right after each head's scale-apply, so S^T matmuls fill the PE
while the h3 norm chain drains.  exp is per-k-tile over valid
columns only; stage2 = col-group-packed rowsums + 1/Z chain (one
strided staging DMA + one merged [P,4*512] broadcast + one scale
mul) + Y^T matmuls with deferred in-place 1/Z scaling; oproj drains
alternate DVE/ACT and the final block's output DMAs are split
across two queues.  Diagonal-block matmuls are causally trimmed.
"""

import math
import sys

for _p in ("/opt/trn_rl_repo",):
    if _p not in sys.path:
        sys.path.append(_p)

from contextlib import ExitStack

import numpy as np
from concourse import bacc, mybir, tile
from concourse.bass_utils import run_bass_kernel_spmd
from concourse.masks import make_identity

B, T, D, H, KVH, DK = 2, 2048, 2048, 16, 4, 128
HPG = H // KVH          # query heads per core (group)
E = HPG * DK            # 512: q-head dims per core
P = 128
TB = 4                  # t blocks of 512
NT = T // P             # 16 tiles of 128 along T
ND = D // P             # 16 contraction tiles
f32 = mybir.dt.float32
bf16 = mybir.dt.bfloat16
fp8 = mybir.dt.float8e4
AF = mybir.ActivationFunctionType
EPS2 = 1e-24


def build_kernel():
    nc = bacc.Bacc(None, target_bir_lowering=False)

    qT8 = nc.declare_dram_parameter("qT8", [TB, P, ND // 2, 2, 512], fp8,
                                    isOutput=False)
    kT8 = nc.declare_dram_parameter("kT8", [ND // 2, P, 2, T], fp8,
                                    isOutput=False)
    vT = nc.declare_dram_parameter("vT", [D, T], bf16, isOutput=False)
    wq8 = nc.declare_dram_parameter("wq8", [P, ND // 2, 2, E], fp8,
                                    isOutput=False)
    wk8 = nc.declare_dram_parameter("wk8", [P, ND // 2, 2, DK], fp8,
                                    isOutput=False)
    wvt = nc.declare_dram_parameter("wvt", [P, ND * DK], bf16, isOutput=False)
    wot = nc.declare_dram_parameter("wot", [P, HPG * D], bf16, isOutput=False)
    gs16 = nc.declare_dram_parameter("gs16", [NT, HPG], f32, isOutput=False)
    outT = nc.declare_dram_parameter("outT", [D, T], bf16,
                                     isOutput=True)

    # DRAM staging for cross-partition rearranges / broadcasts
    n2d = nc.dram_tensor("n2d", [HPG + 1, TB, 512], f32)
    y0d = nc.dram_tensor("y0d", [HPG + 1, T], bf16)
    zd = nc.dram_tensor("zd", [TB, HPG, 512], f32)
    zid = nc.dram_tensor("zid", [TB, HPG * 512], f32)

    with tile.TileContext(nc) as tc:
        with ExitStack() as ctx:
            const = ctx.enter_context(tc.tile_pool(name="const", bufs=1))
            persist = ctx.enter_context(tc.tile_pool(name="persist", bufs=1))
            wkvp = ctx.enter_context(tc.tile_pool(name="wkvp", bufs=1))
            wqop = ctx.enter_context(tc.tile_pool(name="wqop", bufs=1))
            actskv = ctx.enter_context(tc.tile_pool(name="actskv", bufs=4))
            bigA = ctx.enter_context(tc.tile_pool(name="bigA", bufs=4))
            bigB = ctx.enter_context(tc.tile_pool(name="bigB", bufs=4))
            sqp = ctx.enter_context(tc.tile_pool(name="sqp", bufs=1))
            nstage = ctx.enter_context(tc.tile_pool(name="nstage", bufs=1))
            small = ctx.enter_context(tc.tile_pool(name="small", bufs=2))
            bcn = ctx.enter_context(tc.tile_pool(name="bcn", bufs=1))
            bcip = ctx.enter_context(tc.tile_pool(name="bcip", bufs=1))
            ytp = ctx.enter_context(tc.tile_pool(name="ytp", bufs=2))
            ostage = ctx.enter_context(tc.tile_pool(name="ostage", bufs=3))
            psacc = ctx.enter_context(
                tc.tile_pool(name="psacc", bufs=2, space="PSUM"))
            psy = ctx.enter_context(
                tc.tile_pool(name="psy", bufs=2, space="PSUM"))
            pso = ctx.enter_context(
                tc.tile_pool(name="pso", bufs=2, space="PSUM"))

            # ---------------- PE warm-up + constants ----------------
            # PE warm-up runs first on a Pool-memset scratch tile: the
            # HAM clock-gate opens while the first input DMAs stream,
            # with no DVE/iota dependency delaying the first matmul.
            scratch = const.tile([P, 256], bf16, tag="warm")
            nc.gpsimd.memset(scratch[:], 0.001)
            for wi in range(12):
                wps = pso.tile([1, 256], f32, tag="o", name=f"warm{wi}")
                nc.tensor.matmul(wps[:], scratch[:, 0:1], scratch[:],
                                 start=True, stop=True)
            ones_f32 = const.tile([P, 1], f32, tag="ones_f32")
            nc.vector.memset(ones_f32[:], 1.0)
            ones = const.tile([P, 1], bf16, tag="ones")
            nc.vector.tensor_copy(ones[:], ones_f32[:])
            identF = sqp.tile([P, P], f32, tag="sq", name="identF")
            make_identity(nc, identF[:])
            identB = const.tile([P, P], bf16, tag="identB")
            nc.vector.tensor_copy(identB[:], identF[:])
            gs_sb = const.tile([NT, HPG], f32, tag="gs")
            nc.sync.dma_start(gs_sb[:], gs16[:])
            eps16 = const.tile([NT, 1], f32, tag="eps16")
            nc.vector.memset(eps16[:], EPS2)
            # causal keep-mask: M[p, c] = 1.0 iff c >= p + 384.
            # diagonal k-tile j (0..3) of a 512-wide q block uses
            # M[:, 384-128j : 896-128j]  ==  1{ f >= p + 128 j }.
            maskF = sqp.tile([P, 896], f32, tag="sq", name="maskF")
            nc.vector.memset(maskF[:], 1.0)
            nc.gpsimd.affine_select(
                out=maskF[:], in_=maskF[:],
                compare_op=mybir.AluOpType.is_ge,
                fill=0.0, base=-384,
                pattern=[[1, 896]], channel_multiplier=-1,
            )
            maskB = const.tile([P, 896], bf16, tag="maskB")
            nc.vector.tensor_copy(maskB[:], maskF[:])

            qt_sb = persist.tile([P, HPG * T], bf16, tag="qt")
            kt_sb = persist.tile([P, T], bf16, tag="kt")
            vtm_sb = persist.tile([P, T], bf16, tag="vtm")

            def l2norm_scales(xt, idx, gs_col):
                """Column scales rsqrt(sum_d x^2) (* per-head gain) of
                xt [128, T] -> y0d[idx] (DRAM, bf16).  Partition sums via 4
                col-group-packed ones-matmuls; Newton-polished rsqrt in
                [16, 128] layout; staging DMAs on the gpsimd queue."""
                sq = sqp.tile([P, T], bf16, tag="sq")
                nc.vector.tensor_mul(sq[:], xt, xt)
                ps = psy.tile([P, 512], f32, tag="y")
                for tb in range(TB):
                    nc.tensor.matmul(
                        ps[32 * tb:32 * tb + 1, :], ones[:],
                        sq[:, tb * 512:(tb + 1) * 512],
                        start=True, stop=True,
                        tile_position=(0, 32 * tb))
                nfull = nstage.tile([P, 512], f32, tag="nstage")
                nc.vector.tensor_copy(nfull[:], ps[:])
                nc.gpsimd.dma_start(n2d[idx], nfull[0:97:32, :])
                n2c = small.tile([NT, P], f32, tag="n2c")
                nc.gpsimd.dma_start(
                    n2c[:], n2d[idx].rearrange("tb (c p) -> (tb c) p", p=P))
                # DVE-only rsqrt (no ACT Sqrt -> no exp-table thrash):
                # seed y0 = C/n2 with C ~ sqrt(typical n2); n2 is chi^2-
                # concentrated around 128*(0.64^2*2048) so the seed lands
                # within ~40% of 1/sqrt(n2); 3 Newton steps polish it.
                y0 = small.tile([NT, P], f32, tag="y0")
                nc.vector.reciprocal(y0[:], n2c[:])
                nc.vector.tensor_scalar_mul(y0[:], y0[:], 328.0)
                t1 = small.tile([NT, P], f32, tag="t1")
                for _ in range(3):
                    nc.vector.tensor_mul(t1[:], y0[:], y0[:])
                    nc.vector.tensor_mul(t1[:], t1[:], n2c[:])
                    nc.vector.tensor_scalar(
                        out=t1[:], in0=t1[:], scalar1=-0.5, scalar2=1.5,
                        op0=mybir.AluOpType.mult, op1=mybir.AluOpType.add)
                    nc.vector.tensor_mul(y0[:], y0[:], t1[:])
                if gs_col is not None:
                    nc.vector.tensor_mul(
                        y0[:], y0[:], gs_col.to_broadcast((NT, P)))
                y0b = small.tile([NT, P], bf16, tag="y0b")
                nc.vector.tensor_copy(y0b[:], y0[:])
                nc.gpsimd.dma_start(
                    y0d[idx, :].rearrange("(c p) -> c p", p=P), y0b[:])

            def l2norm_apply(xt, idx):
                bc = bcn.tile([P, T], bf16, tag="bc", name=f"bc{idx}")
                nc.scalar.dma_start(
                    bc[:], y0d[idx:idx + 1, :].to_broadcast((P, T)))
                nc.gpsimd.tensor_mul(xt, xt, bc[:])

            # ------------- attention stage defs (used from phase A tail) ----
            def stage1(qb, heads, strips):
                """S^T -> exp -> mask for the given heads of q-block qb."""
                n_k = 4 * (qb + 1)
                pool = bigB if qb % 2 == 0 else bigA
                for h in heads:
                    strip = pool.tile([P, n_k * 512], bf16,
                                      tag=("B" if qb % 2 == 0 else "A"),
                                      name=f"strip{qb}_{h}")
                    qh = qt_sb[:, h * T + qb * 512:h * T + (qb + 1) * 512]
                    for kp in range(n_k // 2):
                        st = psacc.tile([P, 1024], f32, tag="acc",
                                        name=f"st{qb}_{h}_{kp}")
                        for jj in range(2):
                            kt = 2 * kp + jj
                            j = kt - 4 * qb
                            off = 128 * j if j > 0 else 0
                            nc.tensor.matmul(
                                st[:, jj * 512 + off:(jj + 1) * 512],
                                kt_sb[:, kt * P:(kt + 1) * P],
                                qh[:, off:512], start=True, stop=True)
                        # per-kt exp over valid cols only (skips the
                        # sub-diagonal region entirely)
                        for jj in range(2):
                            kt = 2 * kp + jj
                            j = kt - 4 * qb
                            off = 128 * j if j > 0 else 0
                            nc.scalar.activation(
                                strip[:, kt * 512 + off:(kt + 1) * 512],
                                st[:, jj * 512 + off:(jj + 1) * 512],
                                AF.Exp)
                            if j >= 0:
                                nc.vector.tensor_mul(
                                    strip[:, kt * 512 + off:(kt + 1) * 512],
                                    strip[:, kt * 512 + off:(kt + 1) * 512],
                                    maskB[:, 384 - j * P + off:896 - j * P])
                    strips[h] = strip

            def stage2(qb, strips):
                """Packed rowsums + 1/Z chain + Y^T + scaling."""
                n_k = 4 * (qb + 1)
                psz = psy.tile([P, 512], f32, tag="y", name=f"z{qb}")
                for h in range(HPG):
                    for kt in range(n_k):
                        j = kt - 4 * qb
                        off = 128 * j if j > 0 else 0
                        nc.tensor.matmul(
                            psz[32 * h:32 * h + 1, off:512], ones[:],
                            strips[h][:, kt * 512 + off:(kt + 1) * 512],
                            start=(kt == 0), stop=(kt == n_k - 1),
                            tile_position=(0, 32 * h))
                zfull = nstage.tile([P, 512], f32, tag="nstage",
                                    name=f"zfull{qb}")
                with tc.high_priority(offset=200):
                    nc.vector.tensor_copy(zfull[:], psz[:])
                    nc.gpsimd.dma_start(zd[qb], zfull[0:97:32, :])
                zc = small.tile([NT, P], f32, tag="n2c", name=f"zc{qb}")
                with tc.high_priority(offset=200):
                    nc.gpsimd.dma_start(
                        zc[:], zd[qb].rearrange("h (c p) -> (h c) p", p=P))

                ytq = ytp.tile([P, HPG * 512], bf16, tag="ytq",
                               name=f"ytq{qb}")
                # Y matmuls drain PSUM unscaled (no dependence on the 1/Z
                # chain), then the scale is applied in place.
                for h in range(HPG):
                    psyt = psy.tile([P, 512], f32, tag="y",
                                    name=f"yt{qb}_{h}")
                    for kt in range(n_k):
                        j = kt - 4 * qb
                        off = 128 * j if j > 0 else 0
                        nc.tensor.matmul(
                            psyt[:, off:512],
                            vtm_sb[:, kt * P:(kt + 1) * P],
                            strips[h][:, kt * 512 + off:(kt + 1) * 512],
                            start=(kt == 0), stop=(kt == n_k - 1))
                    nc.vector.tensor_copy(
                        ytq[:, h * 512:(h + 1) * 512], psyt[:])
                # 1/Z after the PSUM drains: the reciprocal's wait on the
                # zc DMA must not block the ytq copies in the DVE FIFO.
                zi = small.tile([NT, P], f32, tag="y0", name=f"zi{qb}")
                with tc.high_priority(offset=200):
                    nc.vector.reciprocal(zi[:], zc[:])
                    nc.gpsimd.dma_start(
                        zid[qb].rearrange("(hc p) -> hc p", p=P), zi[:])
                bci = bcip.tile([P, HPG * 512], f32, tag="bci",
                                name=f"bci{qb}")
                with tc.high_priority(offset=200):
                    nc.scalar.dma_start(
                        bci[:],
                        zid[qb:qb + 1, :].to_broadcast((P, HPG * 512)))
                    nc.gpsimd.tensor_mul(ytq[:, :], ytq[:, :], bci[:])
                return ytq

            def oproj(qb, ytq):
                for ot in range(NT):
                    ps = pso.tile([P, 512], f32, tag="o",
                                  name=f"o{qb}_{ot}")
                    for h in range(HPG):
                        nc.tensor.matmul(
                            ps[:],
                            wo_sb[:, h * D + ot * P:h * D + (ot + 1) * P],
                            ytq[:, h * 512:(h + 1) * 512],
                            start=(h == 0), stop=(h == HPG - 1))
                    o_sb = ostage.tile([P, 512], bf16, tag="osb",
                                       name=f"osb{qb}_{ot}")
                    # drains alternate DVE/ACT so neither engine eats the
                    # whole 43us of PSUM->SBUF casts
                    if ot % 2 == 0:
                        nc.vector.tensor_copy(o_sb[:], ps[:])
                    else:
                        nc.scalar.copy(o_sb[:], ps[:])
                    if qb == 3:
                        # tail: split the last block's writes across two
                        # queues so the final DMA isn't one 128KB transfer
                        nc.sync.dma_start(
                            outT[ot * P:(ot + 1) * P,
                                 qb * 512:qb * 512 + 256], o_sb[:, 0:256])
                        nc.scalar.dma_start(
                            outT[ot * P:(ot + 1) * P,
                                 qb * 512 + 256:(qb + 1) * 512],
                            o_sb[:, 256:512])
                    else:
                        nc.sync.dma_start(
                            outT[ot * P:(ot + 1) * P,
                                 qb * 512:(qb + 1) * 512], o_sb[:])

            # ---------------- phase A: projections + norms -------------
            # K first: its norm chain hides under the Q/V projections.
            wk_sb = wkvp.tile([P, ND // 2, 2, DK], fp8, tag="wk",
                              name="wk_sb")
            nc.scalar.dma_start(wk_sb[:], wk8[:])

            def acc4(pfx, pool=None, tag="acc"):
                """4 [128,512] accumulators as halves of 2 PSUM slots."""
                pool = pool or psacc
                a0 = pool.tile([P, 1024], f32, tag=tag, name=f"{pfx}0")
                a1 = pool.tile([P, 1024], f32, tag=tag, name=f"{pfx}1")
                return [a0[:, :512], a0[:, 512:], a1[:, :512], a1[:, 512:]]

            def proj_kv(src_dram, w_sb, dst_fn):
                accs = acc4("acc")
                for n in range(ND):
                    a = actskv.tile([P, T], bf16, tag="akv")
                    nc.sync.dma_start(a[:], src_dram[n * P:(n + 1) * P, :])
                    for tb in range(TB):
                        nc.tensor.matmul(
                            accs[tb],
                            w_sb[:, n * DK:(n + 1) * DK],
                            a[:, tb * 512:(tb + 1) * 512],
                            start=(n == 0), stop=(n == ND - 1))
                for tb in range(TB):
                    dst_fn(tb, accs[tb])

            wq_sb = wqop.tile([P, ND // 2, 2, E], fp8, tag="wqo", name="wq")
            nc.scalar.dma_start(wq_sb[:], wq8[:])
            qa = [bigA.tile([P, ND // 2, 2, 512], fp8, tag="A",
                            name=f"qa{tb}") for tb in range(TB)]

            # K-proj interleaved with Q-proj h0: K accumulates in psacc,
            # Q0 in the idle psy+pso slots.  DMA emission is per-np so
            # kT8[0] transfers immediately and qa8 arrives np-by-np.
            kaccs = acc4("kacc")
            q0accs = [
                psy.tile([P, 512], f32, tag="y", name="q0acc0"),
                psy.tile([P, 512], f32, tag="y", name="q0acc1"),
                pso.tile([P, 512], f32, tag="o", name="q0acc2"),
                pso.tile([P, 512], f32, tag="o", name="q0acc3"),
            ]
            for np_ in range(ND // 2):
                eng0 = nc.scalar if np_ == 0 else nc.sync
                a8 = actskv.tile([P, 2, T], fp8, tag="akv", name=f"k8{np_}")
                eng0.dma_start(a8[:], kT8[np_])
                for tb in range(TB):
                    eng0.dma_start(qa[tb][:, np_, :, :],
                                   qT8[tb, :, np_, :, :])
                for tb in range(TB):
                    nc.tensor.matmul(
                        kaccs[tb], wk_sb[:, np_, :, :],
                        a8[:, :, tb * 512:(tb + 1) * 512],
                        start=(np_ == 0), stop=(np_ == ND // 2 - 1),
                        perf_mode=mybir.MatmulPerfMode.DoubleRow)
                for tb in range(TB):
                    nc.tensor.matmul(
                        q0accs[tb][:],
                        wq_sb[:, np_, :, 0:P],
                        qa[tb][:, np_, :, :],
                        start=(np_ == 0), stop=(np_ == ND // 2 - 1),
                        perf_mode=mybir.MatmulPerfMode.DoubleRow)
            for tb in range(TB):
                nc.vector.tensor_copy(
                    kt_sb[:, tb * 512:(tb + 1) * 512], kaccs[tb])
                nc.vector.tensor_copy(
                    qt_sb[:, tb * 512:(tb + 1) * 512], q0accs[tb][:])
            l2norm_scales(kt_sb[:], HPG, None)
            l2norm_scales(qt_sb[:, 0:T], 0, gs_sb[:, 0:1])

            def proj_q(h):
                accs = acc4(f"qacc{h}_")
                for np_ in range(ND // 2):
                    for tb in range(TB):
                        nc.tensor.matmul(
                            accs[tb],
                            wq_sb[:, np_, :, h * P:(h + 1) * P],
                            qa[tb][:, np_, :, :],
                            start=(np_ == 0), stop=(np_ == ND // 2 - 1),
                            perf_mode=mybir.MatmulPerfMode.DoubleRow)
                for tb in range(TB):
                    nc.vector.tensor_copy(
                        qt_sb[:, h * T + tb * 512:h * T + (tb + 1) * 512],
                        accs[tb])
                l2norm_scales(qt_sb[:, h * T:(h + 1) * T], h,
                              gs_sb[:, h:h + 1])

            l2norm_apply(kt_sb[:], HPG)

            # V projection interleaved with Q-proj h1: the vT DMA streams
            # behind Q compute; V accumulates in the idle pso pool.
            wv_sb = wkvp.tile([P, ND * DK], bf16, tag="wk", name="wv_sb")
            nc.sync.dma_start(wv_sb[:], wvt[:])
            vt_stage = bigB.tile([P, T], bf16, tag="B", name="vt_stage")
            vaccs = acc4("vacc")
            q1accs = [
                psy.tile([P, 512], f32, tag="y", name="q1acc0"),
                psy.tile([P, 512], f32, tag="y", name="q1acc1"),
                pso.tile([P, 512], f32, tag="o", name="q1acc2"),
                pso.tile([P, 512], f32, tag="o", name="q1acc3"),
            ]
            for np_ in range(ND // 2):
                for jj in range(2):
                    n = 2 * np_ + jj
                    a = actskv.tile([P, T], bf16, tag="akv", name=f"v{n}")
                    nc.sync.dma_start(a[:], vT[n * P:(n + 1) * P, :])
                    for tb in range(TB):
                        nc.tensor.matmul(
                            vaccs[tb],
                            wv_sb[:, n * DK:(n + 1) * DK],
                            a[:, tb * 512:(tb + 1) * 512],
                            start=(n == 0), stop=(n == ND - 1))
                for tb in range(TB):
                    nc.tensor.matmul(
                        q1accs[tb][:],
                        wq_sb[:, np_, :, 1 * P:2 * P],
                        qa[tb][:, np_, :, :],
                        start=(np_ == 0), stop=(np_ == ND // 2 - 1),
                        perf_mode=mybir.MatmulPerfMode.DoubleRow)
            for tb in range(TB):
                nc.vector.tensor_copy(
                    vt_stage[:, tb * 512:(tb + 1) * 512], vaccs[tb])
                nc.vector.tensor_copy(
                    qt_sb[:, 1 * T + tb * 512:1 * T + (tb + 1) * 512],
                    q1accs[tb][:])
            l2norm_scales(qt_sb[:, 1 * T:2 * T], 1, gs_sb[:, 1:2])
            l2norm_apply(qt_sb[:, 0 * T:1 * T], 0)
            proj_q(2)
            l2norm_apply(qt_sb[:, 1 * T:2 * T], 1)
            proj_q(3)
            l2norm_apply(qt_sb[:, 2 * T:3 * T], 2)

            # Early per-head stage1 for q-blocks 0/1: S^T matmuls for
            # already-applied heads fill the PE while the h3 norm chain
            # drains; h3's stage1 follows as soon as its apply lands.
            s0, s1, s2, s3 = {}, {}, {}, {}
            stage1(0, [0], s0)
            stage1(1, [0], s1)
            stage1(0, [1], s0)
            stage1(1, [1], s1)
            l2norm_apply(qt_sb[:, 3 * T:4 * T], 3)
            stage1(0, [2], s0)
            stage1(1, [2], s1)
            stage1(0, [3], s0)
            stage1(1, [3], s1)
            # V transposes: PE filler right before attention needs vtm.
            for n in range(NT):
                tp = psy.tile([P, P], bf16, tag="y", name=f"tp{n}")
                nc.tensor.transpose(
                    tp[:], vt_stage[:, n * P:(n + 1) * P], identB[:])
                nc.vector.tensor_copy(vtm_sb[:, n * P:(n + 1) * P], tp[:])

            wo_sb = wqop.tile([P, HPG * D], bf16, tag="wqo", name="wo")
            nc.sync.dma_start(wo_sb[:], wot[:])

            # ------------- phase B: attention + out projection ----------
            ytq0 = stage2(0, s0)
            stage1(2, range(HPG), s2)
            oproj(0, ytq0)
            ytq1 = stage2(1, s1)
            stage1(3, range(HPG), s3)
            oproj(1, ytq1)
            ytq2 = stage2(2, s2)
            ytq3 = stage2(3, s3)
            oproj(2, ytq2)
            oproj(3, ytq3)

    nc.compile()
    return nc


def make_in_maps(q, k, v, Wq, Wk, Wv, Wo, g):
    import ml_dtypes
    st = ml_dtypes.bfloat16
    f8 = ml_dtypes.float8_e4m3
    in_maps = []
    act_t = {}
    for b in range(B):
        qTb = q[b].T  # (D, T)
        # qT8[tb, p, np, j, c] = qT[(2np+j)*128+p, tb*512+c]
        qT8 = np.ascontiguousarray(
            qTb.reshape(ND // 2, 2, P, TB, 512).transpose(3, 2, 0, 1, 4)
        ).astype(f8)
        kT8 = np.ascontiguousarray(
            k[b].T.reshape(ND // 2, 2, P, T).transpose(0, 2, 1, 3)
        ).astype(f8)
        act_t[b] = (
            qT8,
            kT8,
            np.ascontiguousarray(v[b].T).astype(st),
        )

    def wtile(wT, cols):  # wT: (D, cols) -> [P, ND*cols] row-tiled
        return np.ascontiguousarray(
            np.ascontiguousarray(wT).reshape(-1, P, cols)
            .transpose(1, 0, 2).reshape(P, -1)).astype(st)

    def w8tile(wT, cols):  # wT: (D, cols) -> [P, ND//2, 2, cols] fp8 x32
        return np.ascontiguousarray(
            (np.asarray(wT) * 32.0).reshape(ND // 2, 2, P, cols)
            .transpose(2, 0, 1, 3)).astype(f8)

    g_flat = np.asarray(g, dtype=np.float32).reshape(H)
    for c in range(8):
        b, gi = divmod(c, KVH)
        qT8, kT8b, vTb = act_t[b]
        e0 = gi * E
        gvals = g_flat[gi * HPG:(gi + 1) * HPG] / math.sqrt(DK)
        in_maps.append({
            "qT8": qT8, "kT8": kT8b, "vT": vTb,
            "wq8": w8tile(Wq[e0:e0 + E, :].T, E),
            "wk8": w8tile(Wk[gi * DK:(gi + 1) * DK, :].T, DK),
            "wvt": wtile(Wv[gi * DK:(gi + 1) * DK, :].T, DK),
            "wot": wtile(Wo[:, e0:e0 + E].T, D),
            "gs16": np.broadcast_to(gvals[None, :], (NT, HPG)).copy(),
        })
    return in_maps


_cached = {}


def kernel(q, k, v, Wq, Wk, Wv, Wo, g, _trace=False, _tmpdir=None):
    if "nc" not in _cached:
        _cached["nc"] = build_kernel()
    nc = _cached["nc"]
    in_maps = make_in_maps(
        np.asarray(q, np.float32), np.asarray(k, np.float32),
        np.asarray(v, np.float32), np.asarray(Wq, np.float32),
        np.asarray(Wk, np.float32), np.asarray(Wv, np.float32),
        np.asarray(Wo, np.float32), g)
    res = run_bass_kernel_spmd(
        nc, in_maps, list(range(8)), trace=_trace, tmpdir=_tmpdir)
    out = np.empty((B, T, D), dtype=np.float32)
    for b in range(B):
        acc = res.results[4 * b]["outT"].astype(np.float32)
        for gi in range(1, KVH):
            acc += res.results[4 * b + gi]["outT"].astype(np.float32)
        out[b] = acc.T
    kernel.last_results = res
    return out



# revision 22
# speedup vs baseline: 1.0332x; 1.0332x over previous
"""GQA attention with QK-norm for Trainium2, sharded over 8 NeuronCores.

Problem: B=2, T=2048, D=2048, H=16 query heads, KVH=4 kv heads, dk=128.
    Q = q @ Wq.T ; K = k @ Wk.T ; V = v @ Wv.T  (per batch)
    Q = g * l2norm(Q, per head) ; K = l2norm(K, per head)
    out = softmax(causal(Q K^T / sqrt(dk))) V @ Wo.T

Sharding: core c = 4*b + gi handles batch b and kv-head group gi
(4 query heads + 1 kv head). Each core computes a row-shard of the
output projection (O^T partial over its 512 head-dims); the host sums
the 4 partials per batch. No device collectives.

Schedule (v2): PE warm-up runs from t~0 on a Pool-memset scratch
tile.  K-proj runs interleaved with Q-proj head 0 (K in psacc, Q0 in
the psy/pso PSUM slots) with fp8(e4m3)+DoubleRow matmuls (the x32
weight scale cancels in the post-projection l2norm); V-proj (bf16)
interleaves with Q-proj h1 the same way; norm scale chains are
DVE-only (reciprocal + Newton rsqrt, no ACT-table thrash), staged
through DRAM with single strided-partition DMAs.  V transposes stay
on the PE (XBAR DMA-transpose reads are not hazard-tracked by Tile
and race the vt_stage drain).  stage1 for q-blocks 0/1 is emitted
per-head
right after each head's scale-apply, so S^T matmuls fill the PE
while the h3 norm chain drains.  exp is per-k-tile over valid
columns only; stage2 = col-group-packed rowsums + 1/Z chain (one
strided staging DMA + one merged [P,4*512] broadcast + one scale
mul) + Y^T matmuls with deferred in-place 1/Z scaling; oproj drains
alternate DVE/ACT and the final block's output DMAs are split
across two queues.  Diagonal-block matmuls are causally trimmed.
"""

import math
import sys

for _p in ("/opt/trn_rl_repo",):
    if _p not in sys.path:
        sys.path.append(_p)

from contextlib import ExitStack

import numpy as np
from concourse import bacc, mybir, tile
from concourse.bass_utils import run_bass_kernel_spmd
from concourse.masks import make_identity

B, T, D, H, KVH, DK = 2, 2048, 2048, 16, 4, 128
HPG = H // KVH          # query heads per core (group)
E = HPG * DK            # 512: q-head dims per core
P = 128
TB = 4                  # t blocks of 512
NT = T // P             # 16 tiles of 128 along T
ND = D // P             # 16 contraction tiles
f32 = mybir.dt.float32
bf16 = mybir.dt.bfloat16
fp8 = mybir.dt.float8e4
AF = mybir.ActivationFunctionType
EPS2 = 1e-24


def build_kernel():
    nc = bacc.Bacc(None, target_bir_lowering=False)

    qT8 = nc.declare_dram_parameter("qT8", [TB, P, ND // 2, 2, 512], fp8,
                                    isOutput=False)
    kT8 = nc.declare_dram_parameter("kT8", [ND // 2, P, 2, T], fp8,
                                    isOutput=False)
    vT = nc.declare_dram_parameter("vT", [D, T], bf16, isOutput=False)
    wq8 = nc.declare_dram_parameter("wq8", [P, ND // 2, 2, E], fp8,
                                    isOutput=False)
    wk8 = nc.declare_dram_parameter("wk8", [P, ND // 2, 2, DK], fp8,
                                    isOutput=False)
    wvt = nc.declare_dram_parameter("wvt", [P, ND * DK], bf16, isOutput=False)
    wot = nc.declare_dram_parameter("wot", [P, HPG * D], bf16, isOutput=False)
    gs16 = nc.declare_dram_parameter("gs16", [NT, HPG], f32, isOutput=False)
    outT = nc.declare_dram_parameter("outT", [D, T], bf16,
                                     isOutput=True)

    # DRAM staging for cross-partition rearranges / broadcasts
    n2d = nc.dram_tensor("n2d", [HPG + 1, TB, 512], f32)
    y0d = nc.dram_tensor("y0d", [HPG + 1, T], bf16)
    zd = nc.dram_tensor("zd", [TB, HPG, 512], f32)
    zid = nc.dram_tensor("zid", [TB, HPG * 512], bf16)

    with tile.TileContext(nc) as tc:
        with ExitStack() as ctx:
            const = ctx.enter_context(tc.tile_pool(name="const", bufs=1))
            persist = ctx.enter_context(tc.tile_pool(name="persist", bufs=1))
            wkvp = ctx.enter_context(tc.tile_pool(name="wkvp", bufs=1))
            wqop = ctx.enter_context(tc.tile_pool(name="wqop", bufs=1))
            actskv = ctx.enter_context(tc.tile_pool(name="actskv", bufs=4))
            bigA = ctx.enter_context(tc.tile_pool(name="bigA", bufs=4))
            bigB = ctx.enter_context(tc.tile_pool(name="bigB", bufs=4))
            sqp = ctx.enter_context(tc.tile_pool(name="sqp", bufs=1))
            nstage = ctx.enter_context(tc.tile_pool(name="nstage", bufs=1))
            small = ctx.enter_context(tc.tile_pool(name="small", bufs=2))
            bcn = ctx.enter_context(tc.tile_pool(name="bcn", bufs=1))
            bcip = ctx.enter_context(tc.tile_pool(name="bcip", bufs=1))
            ytp = ctx.enter_context(tc.tile_pool(name="ytp", bufs=2))
            ostage = ctx.enter_context(tc.tile_pool(name="ostage", bufs=3))
            psacc = ctx.enter_context(
                tc.tile_pool(name="psacc", bufs=2, space="PSUM"))
            psy = ctx.enter_context(
                tc.tile_pool(name="psy", bufs=2, space="PSUM"))
            pso = ctx.enter_context(
                tc.tile_pool(name="pso", bufs=2, space="PSUM"))

            # ---------------- PE warm-up + constants ----------------
            # PE warm-up runs first on a Pool-memset scratch tile: the
            # HAM clock-gate opens while the first input DMAs stream,
            # with no DVE/iota dependency delaying the first matmul.
            scratch = const.tile([P, 256], bf16, tag="warm")
            nc.gpsimd.memset(scratch[:], 0.001)
            for wi in range(16):
                wps = pso.tile([1, 256], f32, tag="o", name=f"warm{wi}")
                nc.tensor.matmul(wps[:], scratch[:, 0:1], scratch[:],
                                 start=True, stop=True)
            ones_f32 = const.tile([P, 1], f32, tag="ones_f32")
            nc.vector.memset(ones_f32[:], 1.0)
            ones = const.tile([P, 1], bf16, tag="ones")
            nc.vector.tensor_copy(ones[:], ones_f32[:])
            identF = sqp.tile([P, P], f32, tag="sq", name="identF")
            make_identity(nc, identF[:])
            identB = const.tile([P, P], bf16, tag="identB")
            nc.vector.tensor_copy(identB[:], identF[:])
            gs_sb = const.tile([NT, HPG], f32, tag="gs")
            nc.sync.dma_start(gs_sb[:], gs16[:])
            eps16 = const.tile([NT, 1], f32, tag="eps16")
            nc.vector.memset(eps16[:], EPS2)
            # causal keep-mask: M[p, c] = 1.0 iff c >= p + 384.
            # diagonal k-tile j (0..3) of a 512-wide q block uses
            # M[:, 384-128j : 896-128j]  ==  1{ f >= p + 128 j }.
            maskF = sqp.tile([P, 896], f32, tag="sq", name="maskF")
            nc.vector.memset(maskF[:], 1.0)
            nc.gpsimd.affine_select(
                out=maskF[:], in_=maskF[:],
                compare_op=mybir.AluOpType.is_ge,
                fill=0.0, base=-384,
                pattern=[[1, 896]], channel_multiplier=-1,
            )
            maskB = const.tile([P, 896], bf16, tag="maskB")
            nc.vector.tensor_copy(maskB[:], maskF[:])

            qt_sb = persist.tile([P, HPG * T], bf16, tag="qt")
            kt_sb = persist.tile([P, T], bf16, tag="kt")
            vtm_sb = persist.tile([P, T], bf16, tag="vtm")

            def l2norm_scales(xt, idx, gs_col):
                """Column scales rsqrt(sum_d x^2) (* per-head gain) of
                xt [128, T] -> y0d[idx] (DRAM, bf16).  Partition sums via 4
                col-group-packed ones-matmuls; Newton-polished rsqrt in
                [16, 128] layout; staging DMAs on the gpsimd queue."""
                sq = sqp.tile([P, T], bf16, tag="sq")
                nc.vector.tensor_mul(sq[:], xt, xt)
                ps = psy.tile([P, 512], f32, tag="y")
                for tb in range(TB):
                    nc.tensor.matmul(
                        ps[32 * tb:32 * tb + 1, :], ones[:],
                        sq[:, tb * 512:(tb + 1) * 512],
                        start=True, stop=True,
                        tile_position=(0, 32 * tb))
                nfull = nstage.tile([P, 512], f32, tag="nstage")
                nc.vector.tensor_copy(nfull[:], ps[:])
                nc.gpsimd.dma_start(n2d[idx], nfull[0:97:32, :])
                n2c = small.tile([NT, P], f32, tag="n2c")
                nc.gpsimd.dma_start(
                    n2c[:], n2d[idx].rearrange("tb (c p) -> (tb c) p", p=P))
                # DVE-only rsqrt (no ACT Sqrt -> no exp-table thrash):
                # seed y0 = C/n2 with C ~ sqrt(typical n2); n2 is chi^2-
                # concentrated around 128*(0.64^2*2048) so the seed lands
                # within ~40% of 1/sqrt(n2); 3 Newton steps polish it.
                y0 = small.tile([NT, P], f32, tag="y0")
                nc.vector.reciprocal(y0[:], n2c[:])
                nc.vector.tensor_scalar_mul(y0[:], y0[:], 328.0)
                t1 = small.tile([NT, P], f32, tag="t1")
                for _ in range(3):
                    nc.vector.tensor_mul(t1[:], y0[:], y0[:])
                    nc.vector.tensor_mul(t1[:], t1[:], n2c[:])
                    nc.vector.tensor_scalar(
                        out=t1[:], in0=t1[:], scalar1=-0.5, scalar2=1.5,
                        op0=mybir.AluOpType.mult, op1=mybir.AluOpType.add)
                    nc.vector.tensor_mul(y0[:], y0[:], t1[:])
                if gs_col is not None:
                    nc.vector.tensor_mul(
                        y0[:], y0[:], gs_col.to_broadcast((NT, P)))
                y0b = small.tile([NT, P], bf16, tag="y0b")
                nc.vector.tensor_copy(y0b[:], y0[:])
                nc.gpsimd.dma_start(
                    y0d[idx, :].rearrange("(c p) -> c p", p=P), y0b[:])

            def l2norm_apply(xt, idx):
                bc = bcn.tile([P, T], bf16, tag="bc", name=f"bc{idx}")
                nc.scalar.dma_start(
                    bc[:], y0d[idx:idx + 1, :].to_broadcast((P, T)))
                nc.vector.tensor_mul(xt, xt, bc[:])

            # ------------- attention stage defs (used from phase A tail) ----
            def stage1(qb, heads, strips):
                """S^T -> exp -> mask for the given heads of q-block qb."""
                n_k = 4 * (qb + 1)
                pool = bigB if qb % 2 == 0 else bigA
                for h in heads:
                    strip = pool.tile([P, n_k * 512], bf16,
                                      tag=("B" if qb % 2 == 0 else "A"),
                                      name=f"strip{qb}_{h}")
                    qh = qt_sb[:, h * T + qb * 512:h * T + (qb + 1) * 512]
                    for kp in range(n_k // 2):
                        st = psacc.tile([P, 1024], f32, tag="acc",
                                        name=f"st{qb}_{h}_{kp}")
                        for jj in range(2):
                            kt = 2 * kp + jj
                            j = kt - 4 * qb
                            off = 128 * j if j > 0 else 0
                            nc.tensor.matmul(
                                st[:, jj * 512 + off:(jj + 1) * 512],
                                kt_sb[:, kt * P:(kt + 1) * P],
                                qh[:, off:512], start=True, stop=True)
                        # per-kt exp over valid cols only (skips the
                        # sub-diagonal region entirely)
                        for jj in range(2):
                            kt = 2 * kp + jj
                            j = kt - 4 * qb
                            off = 128 * j if j > 0 else 0
                            nc.scalar.activation(
                                strip[:, kt * 512 + off:(kt + 1) * 512],
                                st[:, jj * 512 + off:(jj + 1) * 512],
                                AF.Exp)
                            if j >= 0:
                                nc.vector.tensor_mul(
                                    strip[:, kt * 512 + off:(kt + 1) * 512],
                                    strip[:, kt * 512 + off:(kt + 1) * 512],
                                    maskB[:, 384 - j * P + off:896 - j * P])
                    strips[h] = strip

            def stage2a(qb, strips):
                """Packed rowsums + 1/Z chain + bf16 1/Z broadcast."""
                n_k = 4 * (qb + 1)
                psz = psy.tile([P, 512], f32, tag="y", name=f"z{qb}")
                for h in range(HPG):
                    for kt in range(n_k):
                        j = kt - 4 * qb
                        off = 128 * j if j > 0 else 0
                        nc.tensor.matmul(
                            psz[32 * h:32 * h + 1, off:512], ones[:],
                            strips[h][:, kt * 512 + off:(kt + 1) * 512],
                            start=(kt == 0), stop=(kt == n_k - 1),
                            tile_position=(0, 32 * h))
                zfull = nstage.tile([P, 512], f32, tag="nstage",
                                    name=f"zfull{qb}")
                with tc.high_priority(offset=200):
                    nc.vector.tensor_copy(zfull[:], psz[:])
                    nc.gpsimd.dma_start(zd[qb], zfull[0:97:32, :])
                zc = small.tile([NT, P], f32, tag="n2c", name=f"zc{qb}")
                with tc.high_priority(offset=200):
                    nc.gpsimd.dma_start(
                        zc[:], zd[qb].rearrange("h (c p) -> (h c) p", p=P))
                zi = small.tile([NT, P], f32, tag="y0", name=f"zi{qb}")
                zib = small.tile([NT, P], bf16, tag="y0b", name=f"zib{qb}")
                with tc.high_priority(offset=200):
                    nc.vector.reciprocal(zi[:], zc[:])
                    nc.vector.tensor_copy(zib[:], zi[:])
                    nc.gpsimd.dma_start(
                        zid[qb].rearrange("(hc p) -> hc p", p=P), zib[:])
                bci = bcip.tile([P, HPG * 512], bf16, tag="bci",
                                name=f"bci{qb}")
                with tc.high_priority(offset=200):
                    nc.scalar.dma_start(
                        bci[:],
                        zid[qb:qb + 1, :].to_broadcast((P, HPG * 512)))
                return bci

            def stage2b(qb, strips, bci):
                """Y^T matmuls + deferred in-place 1/Z scaling."""
                n_k = 4 * (qb + 1)
                ytq = ytp.tile([P, HPG * 512], bf16, tag="ytq",
                               name=f"ytq{qb}")
                for h in range(HPG):
                    psyt = psy.tile([P, 512], f32, tag="y",
                                    name=f"yt{qb}_{h}")
                    for kt in range(n_k):
                        j = kt - 4 * qb
                        off = 128 * j if j > 0 else 0
                        nc.tensor.matmul(
                            psyt[:, off:512],
                            vtm_sb[:, kt * P:(kt + 1) * P],
                            strips[h][:, kt * 512 + off:(kt + 1) * 512],
                            start=(kt == 0), stop=(kt == n_k - 1))
                    nc.vector.tensor_copy(
                        ytq[:, h * 512:(h + 1) * 512], psyt[:])
                nc.vector.tensor_mul(ytq[:, :], ytq[:, :], bci[:])
                return ytq

            def oproj(qb, ytq):
                for ot in range(NT):
                    ps = pso.tile([P, 512], f32, tag="o",
                                  name=f"o{qb}_{ot}")
                    for h in range(HPG):
                        nc.tensor.matmul(
                            ps[:],
                            wo_sb[:, h * D + ot * P:h * D + (ot + 1) * P],
                            ytq[:, h * 512:(h + 1) * 512],
                            start=(h == 0), stop=(h == HPG - 1))
                    o_sb = ostage.tile([P, 512], bf16, tag="osb",
                                       name=f"osb{qb}_{ot}")
                    # drains alternate DVE/ACT so neither engine eats the
                    # whole 43us of PSUM->SBUF casts
                    if qb == 3 and ot >= 14:
                        nc.vector.tensor_copy(o_sb[:, 0:256], ps[:, 0:256])
                        nc.scalar.copy(o_sb[:, 256:512], ps[:, 256:512])
                    elif ot % 2 == 0:
                        nc.vector.tensor_copy(o_sb[:], ps[:])
                    else:
                        nc.scalar.copy(o_sb[:], ps[:])
                    if qb == 3:
                        # tail: split the last block's writes across two
                        # queues so the final DMA isn't one 128KB transfer
                        nc.sync.dma_start(
                            outT[ot * P:(ot + 1) * P,
                                 qb * 512:qb * 512 + 256], o_sb[:, 0:256])
                        nc.scalar.dma_start(
                            outT[ot * P:(ot + 1) * P,
                                 qb * 512 + 256:(qb + 1) * 512],
                            o_sb[:, 256:512])
                    else:
                        nc.sync.dma_start(
                            outT[ot * P:(ot + 1) * P,
                                 qb * 512:(qb + 1) * 512], o_sb[:])

            # ---------------- phase A: projections + norms -------------
            # K first: its norm chain hides under the Q/V projections.
            wk_sb = wkvp.tile([P, ND // 2, 2, DK], fp8, tag="wk",
                              name="wk_sb")
            nc.sync.dma_start(wk_sb[:], wk8[:])

            def acc4(pfx, pool=None, tag="acc"):
                """4 [128,512] accumulators as halves of 2 PSUM slots."""
                pool = pool or psacc
                a0 = pool.tile([P, 1024], f32, tag=tag, name=f"{pfx}0")
                a1 = pool.tile([P, 1024], f32, tag=tag, name=f"{pfx}1")
                return [a0[:, :512], a0[:, 512:], a1[:, :512], a1[:, 512:]]

            def proj_kv(src_dram, w_sb, dst_fn):
                accs = acc4("acc")
                for n in range(ND):
                    a = actskv.tile([P, T], bf16, tag="akv")
                    nc.sync.dma_start(a[:], src_dram[n * P:(n + 1) * P, :])
                    for tb in range(TB):
                        nc.tensor.matmul(
                            accs[tb],
                            w_sb[:, n * DK:(n + 1) * DK],
                            a[:, tb * 512:(tb + 1) * 512],
                            start=(n == 0), stop=(n == ND - 1))
                for tb in range(TB):
                    dst_fn(tb, accs[tb])

            wq_sb = wqop.tile([P, ND // 2, 2, E], fp8, tag="wqo", name="wq")
            nc.sync.dma_start(wq_sb[:], wq8[:])
            qa = [bigA.tile([P, ND // 2, 2, 512], fp8, tag="A",
                            name=f"qa{tb}") for tb in range(TB)]

            # K-proj interleaved with Q-proj h0: K accumulates in psacc,
            # Q0 in the idle psy+pso slots.  DMA emission is per-np so
            # kT8[0] transfers immediately and qa8 arrives np-by-np.
            kaccs = acc4("kacc")
            q0accs = [
                psy.tile([P, 512], f32, tag="y", name="q0acc0"),
                psy.tile([P, 512], f32, tag="y", name="q0acc1"),
                pso.tile([P, 512], f32, tag="o", name="q0acc2"),
                pso.tile([P, 512], f32, tag="o", name="q0acc3"),
            ]
            for np_ in range(ND // 2):
                a8 = actskv.tile([P, 2, T], fp8, tag="akv", name=f"k8{np_}")
                nc.sync.dma_start(a8[:], kT8[np_])
                for tb in range(TB):
                    nc.sync.dma_start(qa[tb][:, np_, :, :],
                                      qT8[tb, :, np_, :, :])
                for tb in range(TB):
                    nc.tensor.matmul(
                        kaccs[tb], wk_sb[:, np_, :, :],
                        a8[:, :, tb * 512:(tb + 1) * 512],
                        start=(np_ == 0), stop=(np_ == ND // 2 - 1),
                        perf_mode=mybir.MatmulPerfMode.DoubleRow)
                for tb in range(TB):
                    nc.tensor.matmul(
                        q0accs[tb][:],
                        wq_sb[:, np_, :, 0:P],
                        qa[tb][:, np_, :, :],
                        start=(np_ == 0), stop=(np_ == ND // 2 - 1),
                        perf_mode=mybir.MatmulPerfMode.DoubleRow)
            for tb in range(TB):
                nc.vector.tensor_copy(
                    kt_sb[:, tb * 512:(tb + 1) * 512], kaccs[tb])
                nc.vector.tensor_copy(
                    qt_sb[:, tb * 512:(tb + 1) * 512], q0accs[tb][:])
            l2norm_scales(kt_sb[:], HPG, None)
            l2norm_scales(qt_sb[:, 0:T], 0, gs_sb[:, 0:1])

            def proj_q(h):
                accs = acc4(f"qacc{h}_")
                for np_ in range(ND // 2):
                    for tb in range(TB):
                        nc.tensor.matmul(
                            accs[tb],
                            wq_sb[:, np_, :, h * P:(h + 1) * P],
                            qa[tb][:, np_, :, :],
                            start=(np_ == 0), stop=(np_ == ND // 2 - 1),
                            perf_mode=mybir.MatmulPerfMode.DoubleRow)
                for tb in range(TB):
                    nc.vector.tensor_copy(
                        qt_sb[:, h * T + tb * 512:h * T + (tb + 1) * 512],
                        accs[tb])
                l2norm_scales(qt_sb[:, h * T:(h + 1) * T], h,
                              gs_sb[:, h:h + 1])

            l2norm_apply(kt_sb[:], HPG)

            # V projection interleaved with Q-proj h1: the vT DMA streams
            # behind Q compute; V accumulates in the idle pso pool.
            wv_sb = wkvp.tile([P, ND * DK], bf16, tag="wk", name="wv_sb")
            nc.sync.dma_start(wv_sb[:], wvt[:])
            vt_stage = bigB.tile([P, T], bf16, tag="B", name="vt_stage")
            vaccs = acc4("vacc")
            q1accs = [
                psy.tile([P, 512], f32, tag="y", name="q1acc0"),
                psy.tile([P, 512], f32, tag="y", name="q1acc1"),
                pso.tile([P, 512], f32, tag="o", name="q1acc2"),
                pso.tile([P, 512], f32, tag="o", name="q1acc3"),
            ]
            for np_ in range(ND // 2):
                for jj in range(2):
                    n = 2 * np_ + jj
                    a = actskv.tile([P, T], bf16, tag="akv", name=f"v{n}")
                    nc.sync.dma_start(a[:], vT[n * P:(n + 1) * P, :])
                    for tb in range(TB):
                        nc.tensor.matmul(
                            vaccs[tb],
                            wv_sb[:, n * DK:(n + 1) * DK],
                            a[:, tb * 512:(tb + 1) * 512],
                            start=(n == 0), stop=(n == ND - 1))
                for tb in range(TB):
                    nc.tensor.matmul(
                        q1accs[tb][:],
                        wq_sb[:, np_, :, 1 * P:2 * P],
                        qa[tb][:, np_, :, :],
                        start=(np_ == 0), stop=(np_ == ND // 2 - 1),
                        perf_mode=mybir.MatmulPerfMode.DoubleRow)
            for tb in range(TB):
                nc.vector.tensor_copy(
                    vt_stage[:, tb * 512:(tb + 1) * 512], vaccs[tb])
                nc.vector.tensor_copy(
                    qt_sb[:, 1 * T + tb * 512:1 * T + (tb + 1) * 512],
                    q1accs[tb][:])
            l2norm_scales(qt_sb[:, 1 * T:2 * T], 1, gs_sb[:, 1:2])
            l2norm_apply(qt_sb[:, 0 * T:1 * T], 0)
            proj_q(2)
            l2norm_apply(qt_sb[:, 1 * T:2 * T], 1)
            proj_q(3)
            l2norm_apply(qt_sb[:, 2 * T:3 * T], 2)

            # Early per-head stage1 for q-blocks 0/1: S^T matmuls for
            # already-applied heads fill the PE while the h3 norm chain
            # drains; h3's stage1 follows as soon as its apply lands.
            s0, s1, s2, s3 = {}, {}, {}, {}
            stage1(0, [0], s0)
            stage1(1, [0], s1)
            stage1(0, [1], s0)
            stage1(1, [1], s1)
            l2norm_apply(qt_sb[:, 3 * T:4 * T], 3)
            stage1(0, [2], s0)
            stage1(1, [2], s1)
            stage1(0, [3], s0)
            stage1(1, [3], s1)
            # V transposes: PE filler right before attention needs vtm.
            for n in range(NT):
                tp = psy.tile([P, P], bf16, tag="y", name=f"tp{n}")
                nc.tensor.transpose(
                    tp[:], vt_stage[:, n * P:(n + 1) * P], identB[:])
                nc.vector.tensor_copy(vtm_sb[:, n * P:(n + 1) * P], tp[:])

            wo_sb = wqop.tile([P, HPG * D], bf16, tag="wqo", name="wo")
            nc.sync.dma_start(wo_sb[:], wot[:])

            # ------------- phase B: attention + out projection ----------
            # s2a (rowsums + 1/Z chain) is hoisted so every chain's DMA
            # latency hides under later PE work; s2b (Y + scale) follows.
            bci0 = stage2a(0, s0)
            stage1(2, range(HPG), s2)
            ytq0 = stage2b(0, s0, bci0)
            oproj(0, ytq0)
            bci1 = stage2a(1, s1)
            stage1(3, range(HPG), s3)
            ytq1 = stage2b(1, s1, bci1)
            oproj(1, ytq1)
            bci2 = stage2a(2, s2)
            ytq2 = stage2b(2, s2, bci2)
            bci3 = stage2a(3, s3)
            oproj(2, ytq2)
            ytq3 = stage2b(3, s3, bci3)
            oproj(3, ytq3)

    nc.compile()
    return nc


def make_in_maps(q, k, v, Wq, Wk, Wv, Wo, g):
    import ml_dtypes
    st = ml_dtypes.bfloat16
    f8 = ml_dtypes.float8_e4m3
    in_maps = []
    act_t = {}
    for b in range(B):
        qTb = q[b].T  # (D, T)
        # qT8[tb, p, np, j, c] = qT[(2np+j)*128+p, tb*512+c]
        qT8 = np.ascontiguousarray(
            qTb.reshape(ND // 2, 2, P, TB, 512).transpose(3, 2, 0, 1, 4)
        ).astype(f8)
        kT8 = np.ascontiguousarray(
            k[b].T.reshape(ND // 2, 2, P, T).transpose(0, 2, 1, 3)
        ).astype(f8)
        act_t[b] = (
            qT8,
            kT8,
            np.ascontiguousarray(v[b].T).astype(st),
        )

    def wtile(wT, cols):  # wT: (D, cols) -> [P, ND*cols] row-tiled
        return np.ascontiguousarray(
            np.ascontiguousarray(wT).reshape(-1, P, cols)
            .transpose(1, 0, 2).reshape(P, -1)).astype(st)

    def w8tile(wT, cols):  # wT: (D, cols) -> [P, ND//2, 2, cols] fp8 x32
        return np.ascontiguousarray(
            (np.asarray(wT) * 32.0).reshape(ND // 2, 2, P, cols)
            .transpose(2, 0, 1, 3)).astype(f8)

    g_flat = np.asarray(g, dtype=np.float32).reshape(H)
    for c in range(8):
        b, gi = divmod(c, KVH)
        qT8, kT8b, vTb = act_t[b]
        e0 = gi * E
        gvals = g_flat[gi * HPG:(gi + 1) * HPG] / math.sqrt(DK)
        in_maps.append({
            "qT8": qT8, "kT8": kT8b, "vT": vTb,
            "wq8": w8tile(Wq[e0:e0 + E, :].T, E),
            "wk8": w8tile(Wk[gi * DK:(gi + 1) * DK, :].T, DK),
            "wvt": wtile(Wv[gi * DK:(gi + 1) * DK, :].T, DK),
            "wot": wtile(Wo[:, e0:e0 + E].T, D),
            "gs16": np.broadcast_to(gvals[None, :], (NT, HPG)).copy(),
        })
    return in_maps


_cached = {}


def kernel(q, k, v, Wq, Wk, Wv, Wo, g, _trace=False, _tmpdir=None):
    if "nc" not in _cached:
        _cached["nc"] = build_kernel()
    nc = _cached["nc"]
    in_maps = make_in_maps(
        np.asarray(q, np.float32), np.asarray(k, np.float32),
        np.asarray(v, np.float32), np.asarray(Wq, np.float32),
        np.asarray(Wk, np.float32), np.asarray(Wv, np.float32),
        np.asarray(Wo, np.float32), g)
    res = run_bass_kernel_spmd(
        nc, in_maps, list(range(8)), trace=_trace, tmpdir=_tmpdir)
    out = np.empty((B, T, D), dtype=np.float32)
    for b in range(B):
        acc = res.results[4 * b]["outT"].astype(np.float32)
        for gi in range(1, KVH):
            acc += res.results[4 * b + gi]["outT"].astype(np.float32)
        out[b] = acc.T
    kernel.last_results = res
    return out



# revision 23
# speedup vs baseline: 1.0754x; 1.0409x over previous
"""GQA attention with QK-norm for Trainium2, sharded over 8 NeuronCores.

Problem: B=2, T=2048, D=2048, H=16 query heads, KVH=4 kv heads, dk=128.
    Q = q @ Wq.T ; K = k @ Wk.T ; V = v @ Wv.T  (per batch)
    Q = g * l2norm(Q, per head) ; K = l2norm(K, per head)
    out = softmax(causal(Q K^T / sqrt(dk))) V @ Wo.T

Sharding: core c = 4*b + gi handles batch b and kv-head group gi
(4 query heads + 1 kv head). Each core computes a row-shard of the
output projection (O^T partial over its 512 head-dims); the host sums
the 4 partials per batch. No device collectives.

Schedule (v2): PE warm-up runs from t~0 on a Pool-memset scratch
tile.  K-proj runs interleaved with Q-proj head 0 (K in psacc, Q0 in
the psy/pso PSUM slots) with fp8(e4m3)+DoubleRow matmuls (the x32
weight scale cancels in the post-projection l2norm); V-proj (bf16)
interleaves with Q-proj h1 the same way; norm scale chains are
DVE-only (reciprocal + Newton rsqrt, no ACT-table thrash), staged
through DRAM with single strided-partition DMAs.  V transposes stay
on the PE (XBAR DMA-transpose reads are not hazard-tracked by Tile
and race the vt_stage drain).  stage1 for q-blocks 0/1 is emitted
per-head
right after each head's scale-apply, so S^T matmuls fill the PE
while the h3 norm chain drains.  exp is per-k-tile over valid
columns only; stage2 = col-group-packed rowsums + 1/Z chain (one
strided staging DMA + one merged [P,4*512] broadcast + one scale
mul) + Y^T matmuls with deferred in-place 1/Z scaling; oproj drains
alternate DVE/ACT and the final block's output DMAs are split
across two queues.  Diagonal-block matmuls are causally trimmed.
"""

import math
import sys

for _p in ("/opt/trn_rl_repo",):
    if _p not in sys.path:
        sys.path.append(_p)

from contextlib import ExitStack

import numpy as np
from concourse import bacc, mybir, tile
from concourse.bass_utils import run_bass_kernel_spmd
from concourse.masks import make_identity

B, T, D, H, KVH, DK = 2, 2048, 2048, 16, 4, 128
HPG = H // KVH          # query heads per core (group)
E = HPG * DK            # 512: q-head dims per core
P = 128
TB = 4                  # t blocks of 512
NT = T // P             # 16 tiles of 128 along T
ND = D // P             # 16 contraction tiles
f32 = mybir.dt.float32
bf16 = mybir.dt.bfloat16
fp8 = mybir.dt.float8e4
AF = mybir.ActivationFunctionType
EPS2 = 1e-24


def build_kernel():
    nc = bacc.Bacc(None, target_bir_lowering=False)

    qT8 = nc.declare_dram_parameter("qT8", [TB, P, ND // 2, 2, 512], fp8,
                                    isOutput=False)
    kT8 = nc.declare_dram_parameter("kT8", [ND // 2, P, 2, T], fp8,
                                    isOutput=False)
    vT = nc.declare_dram_parameter("vT", [D, T], bf16, isOutput=False)
    wq8 = nc.declare_dram_parameter("wq8", [P, ND // 2, 2, E], fp8,
                                    isOutput=False)
    wk8 = nc.declare_dram_parameter("wk8", [P, ND // 2, 2, DK], fp8,
                                    isOutput=False)
    wvt = nc.declare_dram_parameter("wvt", [P, ND * DK], bf16, isOutput=False)
    wot = nc.declare_dram_parameter("wot", [P, HPG * D], bf16, isOutput=False)
    gs16 = nc.declare_dram_parameter("gs16", [NT, HPG], f32, isOutput=False)
    outT = nc.declare_dram_parameter("outT", [D, T], bf16,
                                     isOutput=True)

    # DRAM staging for cross-partition rearranges / broadcasts
    n2d = nc.dram_tensor("n2d", [HPG + 1, TB, 512], f32)
    y0d = nc.dram_tensor("y0d", [HPG + 1, T], bf16)
    zd = nc.dram_tensor("zd", [TB, HPG, 512], f32)
    zid = nc.dram_tensor("zid", [TB, HPG * 512], bf16)

    with tile.TileContext(nc) as tc:
        with ExitStack() as ctx:
            const = ctx.enter_context(tc.tile_pool(name="const", bufs=1))
            persist = ctx.enter_context(tc.tile_pool(name="persist", bufs=1))
            wkvp = ctx.enter_context(tc.tile_pool(name="wkvp", bufs=1))
            wqop = ctx.enter_context(tc.tile_pool(name="wqop", bufs=1))
            actskv = ctx.enter_context(tc.tile_pool(name="actskv", bufs=4))
            bigA = ctx.enter_context(tc.tile_pool(name="bigA", bufs=4))
            bigB = ctx.enter_context(tc.tile_pool(name="bigB", bufs=4))
            sqp = ctx.enter_context(tc.tile_pool(name="sqp", bufs=1))
            nstage = ctx.enter_context(tc.tile_pool(name="nstage", bufs=1))
            small = ctx.enter_context(tc.tile_pool(name="small", bufs=2))
            bcn = ctx.enter_context(tc.tile_pool(name="bcn", bufs=1))
            bcip = ctx.enter_context(tc.tile_pool(name="bcip", bufs=1))
            ytp = ctx.enter_context(tc.tile_pool(name="ytp", bufs=2))
            ostage = ctx.enter_context(tc.tile_pool(name="ostage", bufs=3))
            psacc = ctx.enter_context(
                tc.tile_pool(name="psacc", bufs=2, space="PSUM"))
            psy = ctx.enter_context(
                tc.tile_pool(name="psy", bufs=2, space="PSUM"))
            pso = ctx.enter_context(
                tc.tile_pool(name="pso", bufs=2, space="PSUM"))

            # ---------------- PE warm-up + constants ----------------
            # PE warm-up runs first on a Pool-memset scratch tile: the
            # HAM clock-gate opens while the first input DMAs stream,
            # with no DVE/iota dependency delaying the first matmul.
            scratch = const.tile([P, 256], bf16, tag="warm")
            nc.gpsimd.memset(scratch[:], 0.001)
            for wi in range(16):
                wps = pso.tile([1, 256], f32, tag="o", name=f"warm{wi}")
                nc.tensor.matmul(wps[:], scratch[:, 0:1], scratch[:],
                                 start=True, stop=True)
            ones_f32 = const.tile([P, 1], f32, tag="ones_f32")
            nc.vector.memset(ones_f32[:], 1.0)
            ones = const.tile([P, 1], bf16, tag="ones")
            nc.vector.tensor_copy(ones[:], ones_f32[:])
            identF = sqp.tile([P, P], f32, tag="sq", name="identF")
            make_identity(nc, identF[:])
            identB = const.tile([P, P], bf16, tag="identB")
            nc.vector.tensor_copy(identB[:], identF[:])
            gs_sb = const.tile([NT, HPG], f32, tag="gs")
            nc.sync.dma_start(gs_sb[:], gs16[:])
            eps16 = const.tile([NT, 1], f32, tag="eps16")
            nc.vector.memset(eps16[:], EPS2)
            # causal keep-mask: M[p, c] = 1.0 iff c >= p + 384.
            # diagonal k-tile j (0..3) of a 512-wide q block uses
            # M[:, 384-128j : 896-128j]  ==  1{ f >= p + 128 j }.
            maskF = sqp.tile([P, 896], f32, tag="sq", name="maskF")
            nc.vector.memset(maskF[:], 1.0)
            nc.gpsimd.affine_select(
                out=maskF[:], in_=maskF[:],
                compare_op=mybir.AluOpType.is_ge,
                fill=0.0, base=-384,
                pattern=[[1, 896]], channel_multiplier=-1,
            )
            maskB = const.tile([P, 896], bf16, tag="maskB")
            nc.vector.tensor_copy(maskB[:], maskF[:])

            qt_sb = persist.tile([P, HPG * T], bf16, tag="qt")
            kt_sb = persist.tile([P, T], bf16, tag="kt")
            vtm_sb = persist.tile([P, T], bf16, tag="vtm")

            def l2norm_scales(xt, idx, gs_col):
                """Column scales rsqrt(sum_d x^2) (* per-head gain) of
                xt [128, T] -> y0d[idx] (DRAM, bf16).  Partition sums via 4
                col-group-packed ones-matmuls; Newton-polished rsqrt in
                [16, 128] layout; staging DMAs on the gpsimd queue."""
                sq = sqp.tile([P, T], bf16, tag="sq")
                nc.vector.tensor_mul(sq[:], xt, xt)
                ps = psy.tile([P, 512], f32, tag="y")
                for tb in range(TB):
                    nc.tensor.matmul(
                        ps[32 * tb:32 * tb + 1, :], ones[:],
                        sq[:, tb * 512:(tb + 1) * 512],
                        start=True, stop=True,
                        tile_position=(0, 32 * tb))
                nfull = nstage.tile([P, 512], f32, tag="nstage")
                nc.vector.tensor_copy(nfull[:], ps[:])
                nc.gpsimd.dma_start(n2d[idx], nfull[0:97:32, :])
                n2c = small.tile([NT, P], f32, tag="n2c")
                nc.gpsimd.dma_start(
                    n2c[:], n2d[idx].rearrange("tb (c p) -> (tb c) p", p=P))
                # DVE-only rsqrt (no ACT Sqrt -> no exp-table thrash):
                # seed y0 = C/n2 with C ~ sqrt(typical n2); n2 is chi^2-
                # concentrated around 128*(0.64^2*2048) so the seed lands
                # within ~40% of 1/sqrt(n2); 3 Newton steps polish it.
                y0 = small.tile([NT, P], f32, tag="y0")
                nc.vector.reciprocal(y0[:], n2c[:])
                nc.vector.tensor_scalar_mul(y0[:], y0[:], 328.0)
                t1 = small.tile([NT, P], f32, tag="t1")
                for _ in range(3):
                    nc.vector.tensor_mul(t1[:], y0[:], y0[:])
                    nc.vector.tensor_mul(t1[:], t1[:], n2c[:])
                    nc.vector.tensor_scalar(
                        out=t1[:], in0=t1[:], scalar1=-0.5, scalar2=1.5,
                        op0=mybir.AluOpType.mult, op1=mybir.AluOpType.add)
                    nc.vector.tensor_mul(y0[:], y0[:], t1[:])
                if gs_col is not None:
                    nc.vector.tensor_mul(
                        y0[:], y0[:], gs_col.to_broadcast((NT, P)))
                y0b = small.tile([NT, P], bf16, tag="y0b")
                nc.vector.tensor_copy(y0b[:], y0[:])
                nc.gpsimd.dma_start(
                    y0d[idx, :].rearrange("(c p) -> c p", p=P), y0b[:])

            def l2norm_apply(xt, idx):
                bc = bcn.tile([P, T], bf16, tag="bc", name=f"bc{idx}")
                nc.scalar.dma_start(
                    bc[:], y0d[idx:idx + 1, :].to_broadcast((P, T)))
                nc.vector.tensor_mul(xt, xt, bc[:])

            # ------------- attention stage defs (used from phase A tail) ----
            def stage1(qb, heads, strips):
                """S^T -> exp -> mask for the given heads of q-block qb."""
                n_k = 4 * (qb + 1)
                pool = bigB if qb % 2 == 0 else bigA
                for h in heads:
                    strip = pool.tile([P, n_k * 512], bf16,
                                      tag=("B" if qb % 2 == 0 else "A"),
                                      name=f"strip{qb}_{h}")
                    qh = qt_sb[:, h * T + qb * 512:h * T + (qb + 1) * 512]
                    for kp in range(n_k // 2):
                        st = psacc.tile([P, 1024], f32, tag="acc",
                                        name=f"st{qb}_{h}_{kp}")
                        for jj in range(2):
                            kt = 2 * kp + jj
                            j = kt - 4 * qb
                            off = 128 * j if j > 0 else 0
                            nc.tensor.matmul(
                                st[:, jj * 512 + off:(jj + 1) * 512],
                                kt_sb[:, kt * P:(kt + 1) * P],
                                qh[:, off:512], start=True, stop=True)
                        # per-kt exp over valid cols only (skips the
                        # sub-diagonal region entirely)
                        for jj in range(2):
                            kt = 2 * kp + jj
                            j = kt - 4 * qb
                            off = 128 * j if j > 0 else 0
                            nc.scalar.activation(
                                strip[:, kt * 512 + off:(kt + 1) * 512],
                                st[:, jj * 512 + off:(jj + 1) * 512],
                                AF.Exp)
                            if j >= 0:
                                nc.vector.tensor_mul(
                                    strip[:, kt * 512 + off:(kt + 1) * 512],
                                    strip[:, kt * 512 + off:(kt + 1) * 512],
                                    maskB[:, 384 - j * P + off:896 - j * P])
                    strips[h] = strip

            def stage2(qb, strips):
                """Packed rowsums + 1/Z chain + Y^T + scaling."""
                n_k = 4 * (qb + 1)
                psz = psy.tile([P, 512], f32, tag="y", name=f"z{qb}")
                for h in range(HPG):
                    for kt in range(n_k):
                        j = kt - 4 * qb
                        off = 128 * j if j > 0 else 0
                        nc.tensor.matmul(
                            psz[32 * h:32 * h + 1, off:512], ones[:],
                            strips[h][:, kt * 512 + off:(kt + 1) * 512],
                            start=(kt == 0), stop=(kt == n_k - 1),
                            tile_position=(0, 32 * h))
                zfull = nstage.tile([P, 512], f32, tag="nstage",
                                    name=f"zfull{qb}")
                with tc.high_priority(offset=200):
                    nc.vector.tensor_copy(zfull[:], psz[:])
                    nc.gpsimd.dma_start(zd[qb], zfull[0:97:32, :])
                zc = small.tile([NT, P], f32, tag="n2c", name=f"zc{qb}")
                with tc.high_priority(offset=200):
                    nc.gpsimd.dma_start(
                        zc[:], zd[qb].rearrange("h (c p) -> (h c) p", p=P))

                ytq = ytp.tile([P, HPG * 512], bf16, tag="ytq",
                               name=f"ytq{qb}")
                # Y matmuls drain PSUM unscaled (no dependence on the 1/Z
                # chain), then the scale is applied in place.
                for h in range(HPG):
                    psyt = psy.tile([P, 512], f32, tag="y",
                                    name=f"yt{qb}_{h}")
                    for kt in range(n_k):
                        j = kt - 4 * qb
                        off = 128 * j if j > 0 else 0
                        nc.tensor.matmul(
                            psyt[:, off:512],
                            vtm_sb[:, kt * P:(kt + 1) * P],
                            strips[h][:, kt * 512 + off:(kt + 1) * 512],
                            start=(kt == 0), stop=(kt == n_k - 1))
                    nc.vector.tensor_copy(
                        ytq[:, h * 512:(h + 1) * 512], psyt[:])
                # 1/Z after the PSUM drains: the reciprocal's wait on the
                # zc DMA must not block the ytq copies in the DVE FIFO.
                zi = small.tile([NT, P], f32, tag="y0", name=f"zi{qb}")
                zib = small.tile([NT, P], bf16, tag="y0b", name=f"zib{qb}")
                with tc.high_priority(offset=200):
                    nc.vector.reciprocal(zi[:], zc[:])
                    nc.vector.tensor_copy(zib[:], zi[:])
                    nc.gpsimd.dma_start(
                        zid[qb].rearrange("(hc p) -> hc p", p=P), zib[:])
                bci = bcip.tile([P, HPG * 512], bf16, tag="bci",
                                name=f"bci{qb}")
                with tc.high_priority(offset=200):
                    nc.scalar.dma_start(
                        bci[:],
                        zid[qb:qb + 1, :].to_broadcast((P, HPG * 512)))
                    nc.vector.tensor_mul(ytq[:, :], ytq[:, :], bci[:])
                return ytq

            def oproj(qb, ytq):
                for ot in range(NT):
                    ps = pso.tile([P, 512], f32, tag="o",
                                  name=f"o{qb}_{ot}")
                    for h in range(HPG):
                        nc.tensor.matmul(
                            ps[:],
                            wo_sb[:, h * D + ot * P:h * D + (ot + 1) * P],
                            ytq[:, h * 512:(h + 1) * 512],
                            start=(h == 0), stop=(h == HPG - 1))
                    o_sb = ostage.tile([P, 512], bf16, tag="osb",
                                       name=f"osb{qb}_{ot}")
                    # drains alternate DVE/ACT so neither engine eats the
                    # whole 43us of PSUM->SBUF casts
                    if qb == 3 and ot >= 14:
                        nc.vector.tensor_copy(o_sb[:, 0:256], ps[:, 0:256])
                        nc.scalar.copy(o_sb[:, 256:512], ps[:, 256:512])
                    elif ot % 2 == 0:
                        nc.vector.tensor_copy(o_sb[:], ps[:])
                    else:
                        nc.scalar.copy(o_sb[:], ps[:])
                    if qb == 3:
                        # tail: split the last block's writes across two
                        # queues so the final DMA isn't one 128KB transfer
                        nc.sync.dma_start(
                            outT[ot * P:(ot + 1) * P,
                                 qb * 512:qb * 512 + 256], o_sb[:, 0:256])
                        nc.scalar.dma_start(
                            outT[ot * P:(ot + 1) * P,
                                 qb * 512 + 256:(qb + 1) * 512],
                            o_sb[:, 256:512])
                    else:
                        nc.sync.dma_start(
                            outT[ot * P:(ot + 1) * P,
                                 qb * 512:(qb + 1) * 512], o_sb[:])

            # ---------------- phase A: projections + norms -------------
            # K first: its norm chain hides under the Q/V projections.
            wk_sb = wkvp.tile([P, ND // 2, 2, DK], fp8, tag="wk",
                              name="wk_sb")
            nc.sync.dma_start(wk_sb[:], wk8[:])

            def acc4(pfx, pool=None, tag="acc"):
                """4 [128,512] accumulators as halves of 2 PSUM slots."""
                pool = pool or psacc
                a0 = pool.tile([P, 1024], f32, tag=tag, name=f"{pfx}0")
                a1 = pool.tile([P, 1024], f32, tag=tag, name=f"{pfx}1")
                return [a0[:, :512], a0[:, 512:], a1[:, :512], a1[:, 512:]]

            def proj_kv(src_dram, w_sb, dst_fn):
                accs = acc4("acc")
                for n in range(ND):
                    a = actskv.tile([P, T], bf16, tag="akv")
                    nc.sync.dma_start(a[:], src_dram[n * P:(n + 1) * P, :])
                    for tb in range(TB):
                        nc.tensor.matmul(
                            accs[tb],
                            w_sb[:, n * DK:(n + 1) * DK],
                            a[:, tb * 512:(tb + 1) * 512],
                            start=(n == 0), stop=(n == ND - 1))
                for tb in range(TB):
                    dst_fn(tb, accs[tb])

            wq_sb = wqop.tile([P, ND // 2, 2, E], fp8, tag="wqo", name="wq")
            nc.sync.dma_start(wq_sb[:], wq8[:])
            qa = [bigA.tile([P, ND // 2, 2, 512], fp8, tag="A",
                            name=f"qa{tb}") for tb in range(TB)]

            # K-proj interleaved with Q-proj h0: K accumulates in psacc,
            # Q0 in the idle psy+pso slots.  DMA emission is per-np so
            # kT8[0] transfers immediately and qa8 arrives np-by-np.
            kaccs = acc4("kacc")
            q0accs = [
                psy.tile([P, 512], f32, tag="y", name="q0acc0"),
                psy.tile([P, 512], f32, tag="y", name="q0acc1"),
                pso.tile([P, 512], f32, tag="o", name="q0acc2"),
                pso.tile([P, 512], f32, tag="o", name="q0acc3"),
            ]
            for np_ in range(ND // 2):
                a8 = actskv.tile([P, 2, T], fp8, tag="akv", name=f"k8{np_}")
                nc.sync.dma_start(a8[:], kT8[np_])
                for tb in range(TB):
                    nc.sync.dma_start(qa[tb][:, np_, :, :],
                                      qT8[tb, :, np_, :, :])
                for tb in range(TB):
                    nc.tensor.matmul(
                        kaccs[tb], wk_sb[:, np_, :, :],
                        a8[:, :, tb * 512:(tb + 1) * 512],
                        start=(np_ == 0), stop=(np_ == ND // 2 - 1),
                        perf_mode=mybir.MatmulPerfMode.DoubleRow)
                    if np_ == ND // 2 - 1:
                        nc.vector.tensor_copy(
                            kt_sb[:, tb * 512:(tb + 1) * 512], kaccs[tb])
                for tb in range(TB):
                    nc.tensor.matmul(
                        q0accs[tb][:],
                        wq_sb[:, np_, :, 0:P],
                        qa[tb][:, np_, :, :],
                        start=(np_ == 0), stop=(np_ == ND // 2 - 1),
                        perf_mode=mybir.MatmulPerfMode.DoubleRow)
                    if np_ == ND // 2 - 1:
                        nc.vector.tensor_copy(
                            qt_sb[:, tb * 512:(tb + 1) * 512],
                            q0accs[tb][:])
            l2norm_scales(kt_sb[:], HPG, None)
            l2norm_scales(qt_sb[:, 0:T], 0, gs_sb[:, 0:1])

            def proj_q(h):
                accs = acc4(f"qacc{h}_")
                for np_ in range(ND // 2):
                    for tb in range(TB):
                        nc.tensor.matmul(
                            accs[tb],
                            wq_sb[:, np_, :, h * P:(h + 1) * P],
                            qa[tb][:, np_, :, :],
                            start=(np_ == 0), stop=(np_ == ND // 2 - 1),
                            perf_mode=mybir.MatmulPerfMode.DoubleRow)
                for tb in range(TB):
                    nc.vector.tensor_copy(
                        qt_sb[:, h * T + tb * 512:h * T + (tb + 1) * 512],
                        accs[tb])
                l2norm_scales(qt_sb[:, h * T:(h + 1) * T], h,
                              gs_sb[:, h:h + 1])

            l2norm_apply(kt_sb[:], HPG)

            # V projection interleaved with Q-proj h1: the vT DMA streams
            # behind Q compute; V accumulates in the idle pso pool.
            wv_sb = wkvp.tile([P, ND * DK], bf16, tag="wk", name="wv_sb")
            nc.sync.dma_start(wv_sb[:], wvt[:])
            vt_stage = bigB.tile([P, T], bf16, tag="B", name="vt_stage")
            vaccs = acc4("vacc")
            q1accs = [
                psy.tile([P, 512], f32, tag="y", name="q1acc0"),
                psy.tile([P, 512], f32, tag="y", name="q1acc1"),
                pso.tile([P, 512], f32, tag="o", name="q1acc2"),
                pso.tile([P, 512], f32, tag="o", name="q1acc3"),
            ]
            for np_ in range(ND // 2):
                for jj in range(2):
                    n = 2 * np_ + jj
                    a = actskv.tile([P, T], bf16, tag="akv", name=f"v{n}")
                    nc.sync.dma_start(a[:], vT[n * P:(n + 1) * P, :])
                    for tb in range(TB):
                        nc.tensor.matmul(
                            vaccs[tb],
                            wv_sb[:, n * DK:(n + 1) * DK],
                            a[:, tb * 512:(tb + 1) * 512],
                            start=(n == 0), stop=(n == ND - 1))
                        if n == ND - 1:
                            nc.vector.tensor_copy(
                                vt_stage[:, tb * 512:(tb + 1) * 512],
                                vaccs[tb])
                for tb in range(TB):
                    nc.tensor.matmul(
                        q1accs[tb][:],
                        wq_sb[:, np_, :, 1 * P:2 * P],
                        qa[tb][:, np_, :, :],
                        start=(np_ == 0), stop=(np_ == ND // 2 - 1),
                        perf_mode=mybir.MatmulPerfMode.DoubleRow)
                    if np_ == ND // 2 - 1:
                        nc.vector.tensor_copy(
                            qt_sb[:, 1 * T + tb * 512:
                                  1 * T + (tb + 1) * 512],
                            q1accs[tb][:])
            l2norm_scales(qt_sb[:, 1 * T:2 * T], 1, gs_sb[:, 1:2])
            l2norm_apply(qt_sb[:, 0 * T:1 * T], 0)
            proj_q(2)
            l2norm_apply(qt_sb[:, 1 * T:2 * T], 1)
            proj_q(3)
            l2norm_apply(qt_sb[:, 2 * T:3 * T], 2)

            # Early per-head stage1 for q-blocks 0/1: S^T matmuls for
            # already-applied heads fill the PE while the h3 norm chain
            # drains; h3's stage1 follows as soon as its apply lands.
            s0, s1, s2, s3 = {}, {}, {}, {}
            stage1(0, [0], s0)
            stage1(1, [0], s1)
            stage1(0, [1], s0)
            stage1(1, [1], s1)
            l2norm_apply(qt_sb[:, 3 * T:4 * T], 3)
            stage1(0, [2], s0)
            stage1(1, [2], s1)
            stage1(0, [3], s0)
            stage1(1, [3], s1)
            # V transposes: PE filler right before attention needs vtm.
            for n in range(NT):
                tp = psy.tile([P, P], bf16, tag="y", name=f"tp{n}")
                nc.tensor.transpose(
                    tp[:], vt_stage[:, n * P:(n + 1) * P], identB[:])
                nc.vector.tensor_copy(vtm_sb[:, n * P:(n + 1) * P], tp[:])

            wo_sb = wqop.tile([P, HPG * D], bf16, tag="wqo", name="wo")
            nc.sync.dma_start(wo_sb[:], wot[:])

            # ------------- phase B: attention + out projection ----------
            ytq0 = stage2(0, s0)
            stage1(2, range(HPG), s2)
            oproj(0, ytq0)
            ytq1 = stage2(1, s1)
            stage1(3, range(HPG), s3)
            oproj(1, ytq1)
            ytq2 = stage2(2, s2)
            ytq3 = stage2(3, s3)
            oproj(2, ytq2)
            oproj(3, ytq3)

    nc.compile()
    return nc


def make_in_maps(q, k, v, Wq, Wk, Wv, Wo, g):
    import ml_dtypes
    st = ml_dtypes.bfloat16
    f8 = ml_dtypes.float8_e4m3
    in_maps = []
    act_t = {}
    for b in range(B):
        qTb = q[b].T  # (D, T)
        # qT8[tb, p, np, j, c] = qT[(2np+j)*128+p, tb*512+c]
        qT8 = np.ascontiguousarray(
            qTb.reshape(ND // 2, 2, P, TB, 512).transpose(3, 2, 0, 1, 4)
        ).astype(f8)
        kT8 = np.ascontiguousarray(
            k[b].T.reshape(ND // 2, 2, P, T).transpose(0, 2, 1, 3)
        ).astype(f8)
        act_t[b] = (
            qT8,
            kT8,
            np.ascontiguousarray(v[b].T).astype(st),
        )

    def wtile(wT, cols):  # wT: (D, cols) -> [P, ND*cols] row-tiled
        return np.ascontiguousarray(
            np.ascontiguousarray(wT).reshape(-1, P, cols)
            .transpose(1, 0, 2).reshape(P, -1)).astype(st)

    def w8tile(wT, cols):  # wT: (D, cols) -> [P, ND//2, 2, cols] fp8 x32
        return np.ascontiguousarray(
            (np.asarray(wT) * 32.0).reshape(ND // 2, 2, P, cols)
            .transpose(2, 0, 1, 3)).astype(f8)

    g_flat = np.asarray(g, dtype=np.float32).reshape(H)
    for c in range(8):
        b, gi = divmod(c, KVH)
        qT8, kT8b, vTb = act_t[b]
        e0 = gi * E
        gvals = g_flat[gi * HPG:(gi + 1) * HPG] / math.sqrt(DK)
        in_maps.append({
            "qT8": qT8, "kT8": kT8b, "vT": vTb,
            "wq8": w8tile(Wq[e0:e0 + E, :].T, E),
            "wk8": w8tile(Wk[gi * DK:(gi + 1) * DK, :].T, DK),
            "wvt": wtile(Wv[gi * DK:(gi + 1) * DK, :].T, DK),
            "wot": wtile(Wo[:, e0:e0 + E].T, D),
            "gs16": np.broadcast_to(gvals[None, :], (NT, HPG)).copy(),
        })
    return in_maps


_cached = {}


def kernel(q, k, v, Wq, Wk, Wv, Wo, g, _trace=False, _tmpdir=None):
    if "nc" not in _cached:
        _cached["nc"] = build_kernel()
    nc = _cached["nc"]
    in_maps = make_in_maps(
        np.asarray(q, np.float32), np.asarray(k, np.float32),
        np.asarray(v, np.float32), np.asarray(Wq, np.float32),
        np.asarray(Wk, np.float32), np.asarray(Wv, np.float32),
        np.asarray(Wo, np.float32), g)
    res = run_bass_kernel_spmd(
        nc, in_maps, list(range(8)), trace=_trace, tmpdir=_tmpdir)
    out = np.empty((B, T, D), dtype=np.float32)
    for b in range(B):
        acc = res.results[4 * b]["outT"].astype(np.float32)
        for gi in range(1, KVH):
            acc += res.results[4 * b + gi]["outT"].astype(np.float32)
        out[b] = acc.T
    kernel.last_results = res
    return out



# revision 24
# speedup vs baseline: 1.0805x; 1.0047x over previous
"""GQA attention with QK-norm for Trainium2, sharded over 8 NeuronCores.

Problem: B=2, T=2048, D=2048, H=16 query heads, KVH=4 kv heads, dk=128.
    Q = q @ Wq.T ; K = k @ Wk.T ; V = v @ Wv.T  (per batch)
    Q = g * l2norm(Q, per head) ; K = l2norm(K, per head)
    out = softmax(causal(Q K^T / sqrt(dk))) V @ Wo.T

Sharding: core c = 4*b + gi handles batch b and kv-head group gi
(4 query heads + 1 kv head). Each core computes a row-shard of the
output projection (O^T partial over its 512 head-dims); the host sums
the 4 partials per batch. No device collectives.

Schedule (v2): PE warm-up runs from t~0 on a Pool-memset scratch
tile.  K-proj runs interleaved with Q-proj head 0 (K in psacc, Q0 in
the psy/pso PSUM slots) with fp8(e4m3)+DoubleRow matmuls (the x32
weight scale cancels in the post-projection l2norm); V-proj (bf16)
interleaves with Q-proj h1 the same way; norm scale chains are
DVE-only (reciprocal + Newton rsqrt, no ACT-table thrash), staged
through DRAM with single strided-partition DMAs.  V transposes stay
on the PE (XBAR DMA-transpose reads are not hazard-tracked by Tile
and race the vt_stage drain).  stage1 for q-blocks 0/1 is emitted
per-head
right after each head's scale-apply, so S^T matmuls fill the PE
while the h3 norm chain drains.  exp is per-k-tile over valid
columns only; stage2 = col-group-packed rowsums + 1/Z chain (one
strided staging DMA + one merged [P,4*512] broadcast + one scale
mul) + Y^T matmuls with deferred in-place 1/Z scaling; oproj drains
alternate DVE/ACT and the final block's output DMAs are split
across two queues.  Diagonal-block matmuls are causally trimmed.
"""

import math
import sys

for _p in ("/opt/trn_rl_repo",):
    if _p not in sys.path:
        sys.path.append(_p)

from contextlib import ExitStack

import numpy as np
from concourse import bacc, mybir, tile
from concourse.bass_utils import run_bass_kernel_spmd
from concourse.masks import make_identity

B, T, D, H, KVH, DK = 2, 2048, 2048, 16, 4, 128
HPG = H // KVH          # query heads per core (group)
E = HPG * DK            # 512: q-head dims per core
P = 128
TB = 4                  # t blocks of 512
NT = T // P             # 16 tiles of 128 along T
ND = D // P             # 16 contraction tiles
f32 = mybir.dt.float32
bf16 = mybir.dt.bfloat16
fp8 = mybir.dt.float8e4
AF = mybir.ActivationFunctionType
EPS2 = 1e-24


def build_kernel():
    nc = bacc.Bacc(None, target_bir_lowering=False)

    qT8 = nc.declare_dram_parameter("qT8", [TB, P, ND // 2, 2, 512], fp8,
                                    isOutput=False)
    kT8 = nc.declare_dram_parameter("kT8", [ND // 2, P, 2, T], fp8,
                                    isOutput=False)
    vT = nc.declare_dram_parameter("vT", [D, T], bf16, isOutput=False)
    wq8 = nc.declare_dram_parameter("wq8", [P, ND // 2, 2, E], fp8,
                                    isOutput=False)
    wk8 = nc.declare_dram_parameter("wk8", [P, ND // 2, 2, DK], fp8,
                                    isOutput=False)
    wvt = nc.declare_dram_parameter("wvt", [P, ND * DK], bf16, isOutput=False)
    wot = nc.declare_dram_parameter("wot", [P, HPG * D], bf16, isOutput=False)
    gs16 = nc.declare_dram_parameter("gs16", [NT, HPG], f32, isOutput=False)
    outT = nc.declare_dram_parameter("outT", [D, T], bf16,
                                     isOutput=True)

    # DRAM staging for cross-partition rearranges / broadcasts
    n2d = nc.dram_tensor("n2d", [HPG + 1, TB, 512], f32)
    y0d = nc.dram_tensor("y0d", [HPG + 1, T], bf16)
    zd = nc.dram_tensor("zd", [TB, HPG, 512], f32)
    zid = nc.dram_tensor("zid", [TB, HPG * 512], bf16)

    with tile.TileContext(nc) as tc:
        with ExitStack() as ctx:
            const = ctx.enter_context(tc.tile_pool(name="const", bufs=1))
            persist = ctx.enter_context(tc.tile_pool(name="persist", bufs=1))
            wkvp = ctx.enter_context(tc.tile_pool(name="wkvp", bufs=1))
            wqop = ctx.enter_context(tc.tile_pool(name="wqop", bufs=1))
            actskv = ctx.enter_context(tc.tile_pool(name="actskv", bufs=4))
            bigA = ctx.enter_context(tc.tile_pool(name="bigA", bufs=4))
            bigB = ctx.enter_context(tc.tile_pool(name="bigB", bufs=4))
            sqp = ctx.enter_context(tc.tile_pool(name="sqp", bufs=1))
            nstage = ctx.enter_context(tc.tile_pool(name="nstage", bufs=1))
            small = ctx.enter_context(tc.tile_pool(name="small", bufs=2))
            bcn = ctx.enter_context(tc.tile_pool(name="bcn", bufs=1))
            bcip = ctx.enter_context(tc.tile_pool(name="bcip", bufs=1))
            ytp = ctx.enter_context(tc.tile_pool(name="ytp", bufs=2))
            ostage = ctx.enter_context(tc.tile_pool(name="ostage", bufs=3))
            psacc = ctx.enter_context(
                tc.tile_pool(name="psacc", bufs=2, space="PSUM"))
            psy = ctx.enter_context(
                tc.tile_pool(name="psy", bufs=2, space="PSUM"))
            pso = ctx.enter_context(
                tc.tile_pool(name="pso", bufs=2, space="PSUM"))

            # ---------------- PE warm-up + constants ----------------
            # PE warm-up runs first on a Pool-memset scratch tile: the
            # HAM clock-gate opens while the first input DMAs stream,
            # with no DVE/iota dependency delaying the first matmul.
            scratch = const.tile([P, 256], bf16, tag="warm")
            nc.gpsimd.memset(scratch[:], 0.001)
            for wi in range(16):
                wps = pso.tile([1, 256], f32, tag="o", name=f"warm{wi}")
                nc.tensor.matmul(wps[:], scratch[:, 0:1], scratch[:],
                                 start=True, stop=True)
            ones_f32 = const.tile([P, 1], f32, tag="ones_f32")
            nc.vector.memset(ones_f32[:], 1.0)
            ones = const.tile([P, 1], bf16, tag="ones")
            nc.vector.tensor_copy(ones[:], ones_f32[:])
            identF = sqp.tile([P, P], f32, tag="sq", name="identF")
            make_identity(nc, identF[:])
            identB = const.tile([P, P], bf16, tag="identB")
            nc.vector.tensor_copy(identB[:], identF[:])
            gs_sb = const.tile([NT, HPG], f32, tag="gs")
            nc.sync.dma_start(gs_sb[:], gs16[:])
            eps16 = const.tile([NT, 1], f32, tag="eps16")
            nc.vector.memset(eps16[:], EPS2)
            # causal keep-mask: M[p, c] = 1.0 iff c >= p + 384.
            # diagonal k-tile j (0..3) of a 512-wide q block uses
            # M[:, 384-128j : 896-128j]  ==  1{ f >= p + 128 j }.
            maskF = sqp.tile([P, 896], f32, tag="sq", name="maskF")
            nc.vector.memset(maskF[:], 1.0)
            nc.gpsimd.affine_select(
                out=maskF[:], in_=maskF[:],
                compare_op=mybir.AluOpType.is_ge,
                fill=0.0, base=-384,
                pattern=[[1, 896]], channel_multiplier=-1,
            )
            maskB = const.tile([P, 896], bf16, tag="maskB")
            nc.vector.tensor_copy(maskB[:], maskF[:])

            qt_sb = persist.tile([P, HPG * T], bf16, tag="qt")
            kt_sb = persist.tile([P, T], bf16, tag="kt")
            vtm_sb = persist.tile([P, T], bf16, tag="vtm")

            def l2norm_scales(xt, idx, gs_col):
                """Column scales rsqrt(sum_d x^2) (* per-head gain) of
                xt [128, T] -> y0d[idx] (DRAM, bf16).  Partition sums via 4
                col-group-packed ones-matmuls; Newton-polished rsqrt in
                [16, 128] layout; staging DMAs on the gpsimd queue."""
                sq = sqp.tile([P, T], bf16, tag="sq")
                nc.vector.tensor_mul(sq[:], xt, xt)
                ps = psy.tile([P, 512], f32, tag="y")
                for tb in range(TB):
                    nc.tensor.matmul(
                        ps[32 * tb:32 * tb + 1, :], ones[:],
                        sq[:, tb * 512:(tb + 1) * 512],
                        start=True, stop=True,
                        tile_position=(0, 32 * tb))
                nfull = nstage.tile([P, 512], f32, tag="nstage")
                nc.vector.tensor_copy(nfull[:], ps[:])
                nc.gpsimd.dma_start(n2d[idx], nfull[0:97:32, :])
                n2c = small.tile([NT, P], f32, tag="n2c")
                nc.gpsimd.dma_start(
                    n2c[:], n2d[idx].rearrange("tb (c p) -> (tb c) p", p=P))
                # DVE-only rsqrt (no ACT Sqrt -> no exp-table thrash):
                # seed y0 = C/n2 with C ~ sqrt(typical n2); n2 is chi^2-
                # concentrated around 128*(0.64^2*2048) so the seed lands
                # within ~40% of 1/sqrt(n2); 3 Newton steps polish it.
                y0 = small.tile([NT, P], f32, tag="y0")
                nc.vector.reciprocal(y0[:], n2c[:])
                nc.vector.tensor_scalar_mul(y0[:], y0[:], 328.0)
                t1 = small.tile([NT, P], f32, tag="t1")
                for _ in range(3):
                    nc.vector.tensor_mul(t1[:], y0[:], y0[:])
                    nc.vector.tensor_mul(t1[:], t1[:], n2c[:])
                    nc.vector.tensor_scalar(
                        out=t1[:], in0=t1[:], scalar1=-0.5, scalar2=1.5,
                        op0=mybir.AluOpType.mult, op1=mybir.AluOpType.add)
                    nc.vector.tensor_mul(y0[:], y0[:], t1[:])
                if gs_col is not None:
                    nc.vector.tensor_mul(
                        y0[:], y0[:], gs_col.to_broadcast((NT, P)))
                y0b = small.tile([NT, P], bf16, tag="y0b")
                nc.vector.tensor_copy(y0b[:], y0[:])
                nc.gpsimd.dma_start(
                    y0d[idx, :].rearrange("(c p) -> c p", p=P), y0b[:])

            def l2norm_apply(xt, idx):
                bc = bcn.tile([P, T], bf16, tag="bc", name=f"bc{idx}")
                nc.scalar.dma_start(
                    bc[:], y0d[idx:idx + 1, :].to_broadcast((P, T)))
                nc.vector.tensor_mul(xt, xt, bc[:])

            # ------------- attention stage defs (used from phase A tail) ----
            def stage1(qb, heads, strips):
                """S^T -> exp -> mask for the given heads of q-block qb."""
                n_k = 4 * (qb + 1)
                pool = bigB if qb % 2 == 0 else bigA
                for h in heads:
                    strip = pool.tile([P, n_k * 512], bf16,
                                      tag=("B" if qb % 2 == 0 else "A"),
                                      name=f"strip{qb}_{h}")
                    qh = qt_sb[:, h * T + qb * 512:h * T + (qb + 1) * 512]
                    for kp in range(n_k // 2):
                        st = psacc.tile([P, 1024], f32, tag="acc",
                                        name=f"st{qb}_{h}_{kp}")
                        for jj in range(2):
                            kt = 2 * kp + jj
                            j = kt - 4 * qb
                            off = 128 * j if j > 0 else 0
                            nc.tensor.matmul(
                                st[:, jj * 512 + off:(jj + 1) * 512],
                                kt_sb[:, kt * P:(kt + 1) * P],
                                qh[:, off:512], start=True, stop=True)
                        # per-kt exp over valid cols only (skips the
                        # sub-diagonal region entirely)
                        for jj in range(2):
                            kt = 2 * kp + jj
                            j = kt - 4 * qb
                            off = 128 * j if j > 0 else 0
                            nc.scalar.activation(
                                strip[:, kt * 512 + off:(kt + 1) * 512],
                                st[:, jj * 512 + off:(jj + 1) * 512],
                                AF.Exp)
                            if j >= 0:
                                nc.vector.tensor_mul(
                                    strip[:, kt * 512 + off:(kt + 1) * 512],
                                    strip[:, kt * 512 + off:(kt + 1) * 512],
                                    maskB[:, 384 - j * P + off:896 - j * P])
                    strips[h] = strip

            def stage2(qb, strips, early_recip=False):
                """Packed rowsums + 1/Z chain + Y^T + scaling."""
                n_k = 4 * (qb + 1)
                psz = psy.tile([P, 512], f32, tag="y", name=f"z{qb}")
                for h in range(HPG):
                    for kt in range(n_k):
                        j = kt - 4 * qb
                        off = 128 * j if j > 0 else 0
                        nc.tensor.matmul(
                            psz[32 * h:32 * h + 1, off:512], ones[:],
                            strips[h][:, kt * 512 + off:(kt + 1) * 512],
                            start=(kt == 0), stop=(kt == n_k - 1),
                            tile_position=(0, 32 * h))
                zfull = nstage.tile([P, 512], f32, tag="nstage",
                                    name=f"zfull{qb}")
                with tc.high_priority(offset=200):
                    nc.vector.tensor_copy(zfull[:], psz[:])
                    nc.gpsimd.dma_start(zd[qb], zfull[0:97:32, :])
                zc = small.tile([NT, P], f32, tag="n2c", name=f"zc{qb}")
                with tc.high_priority(offset=200):
                    nc.gpsimd.dma_start(
                        zc[:], zd[qb].rearrange("h (c p) -> (h c) p", p=P))

                def zchain():
                    zi = small.tile([NT, P], f32, tag="y0", name=f"zi{qb}")
                    zib = small.tile([NT, P], bf16, tag="y0b",
                                     name=f"zib{qb}")
                    with tc.high_priority(offset=200):
                        nc.vector.reciprocal(zi[:], zc[:])
                        nc.vector.tensor_copy(zib[:], zi[:])
                        nc.gpsimd.dma_start(
                            zid[qb].rearrange("(hc p) -> hc p", p=P),
                            zib[:])
                    bci = bcip.tile([P, HPG * 512], bf16, tag="bci",
                                    name=f"bci{qb}")
                    with tc.high_priority(offset=200):
                        nc.scalar.dma_start(
                            bci[:],
                            zid[qb:qb + 1, :].to_broadcast((P, HPG * 512)))
                    return bci

                # For the last q-block the recip's zc wait in the DVE FIFO
                # is covered by oproj(qb-1), so hoisting it buys an earlier
                # 1/Z broadcast and a shorter tail.
                bci = zchain() if early_recip else None

                ytq = ytp.tile([P, HPG * 512], bf16, tag="ytq",
                               name=f"ytq{qb}")
                # Y matmuls drain PSUM unscaled (no dependence on the 1/Z
                # chain), then the scale is applied in place.
                for h in range(HPG):
                    psyt = psy.tile([P, 512], f32, tag="y",
                                    name=f"yt{qb}_{h}")
                    for kt in range(n_k):
                        j = kt - 4 * qb
                        off = 128 * j if j > 0 else 0
                        nc.tensor.matmul(
                            psyt[:, off:512],
                            vtm_sb[:, kt * P:(kt + 1) * P],
                            strips[h][:, kt * 512 + off:(kt + 1) * 512],
                            start=(kt == 0), stop=(kt == n_k - 1))
                    nc.vector.tensor_copy(
                        ytq[:, h * 512:(h + 1) * 512], psyt[:])
                # 1/Z after the PSUM drains: the reciprocal's wait on the
                # zc DMA must not block the ytq copies in the DVE FIFO.
                if bci is None:
                    bci = zchain()
                nc.vector.tensor_mul(ytq[:, :], ytq[:, :], bci[:])
                return ytq

            def oproj(qb, ytq):
                for ot in range(NT):
                    ps = pso.tile([P, 512], f32, tag="o",
                                  name=f"o{qb}_{ot}")
                    for h in range(HPG):
                        nc.tensor.matmul(
                            ps[:],
                            wo_sb[:, h * D + ot * P:h * D + (ot + 1) * P],
                            ytq[:, h * 512:(h + 1) * 512],
                            start=(h == 0), stop=(h == HPG - 1))
                    o_sb = ostage.tile([P, 512], bf16, tag="osb",
                                       name=f"osb{qb}_{ot}")
                    # drains alternate DVE/ACT so neither engine eats the
                    # whole 43us of PSUM->SBUF casts
                    if qb == 3:
                        nc.vector.tensor_copy(o_sb[:, 0:256], ps[:, 0:256])
                        nc.scalar.copy(o_sb[:, 256:512], ps[:, 256:512])
                    elif ot % 2 == 0:
                        nc.vector.tensor_copy(o_sb[:], ps[:])
                    else:
                        nc.scalar.copy(o_sb[:], ps[:])
                    if qb == 3:
                        # tail: split the last block's writes across two
                        # queues so the final DMA isn't one 128KB transfer
                        nc.sync.dma_start(
                            outT[ot * P:(ot + 1) * P,
                                 qb * 512:qb * 512 + 256], o_sb[:, 0:256])
                        nc.scalar.dma_start(
                            outT[ot * P:(ot + 1) * P,
                                 qb * 512 + 256:(qb + 1) * 512],
                            o_sb[:, 256:512])
                    else:
                        nc.sync.dma_start(
                            outT[ot * P:(ot + 1) * P,
                                 qb * 512:(qb + 1) * 512], o_sb[:])

            # ---------------- phase A: projections + norms -------------
            # K first: its norm chain hides under the Q/V projections.
            wk_sb = wkvp.tile([P, ND // 2, 2, DK], fp8, tag="wk",
                              name="wk_sb")
            nc.sync.dma_start(wk_sb[:], wk8[:])

            def acc4(pfx, pool=None, tag="acc"):
                """4 [128,512] accumulators as halves of 2 PSUM slots."""
                pool = pool or psacc
                a0 = pool.tile([P, 1024], f32, tag=tag, name=f"{pfx}0")
                a1 = pool.tile([P, 1024], f32, tag=tag, name=f"{pfx}1")
                return [a0[:, :512], a0[:, 512:], a1[:, :512], a1[:, 512:]]

            def proj_kv(src_dram, w_sb, dst_fn):
                accs = acc4("acc")
                for n in range(ND):
                    a = actskv.tile([P, T], bf16, tag="akv")
                    nc.sync.dma_start(a[:], src_dram[n * P:(n + 1) * P, :])
                    for tb in range(TB):
                        nc.tensor.matmul(
                            accs[tb],
                            w_sb[:, n * DK:(n + 1) * DK],
                            a[:, tb * 512:(tb + 1) * 512],
                            start=(n == 0), stop=(n == ND - 1))
                for tb in range(TB):
                    dst_fn(tb, accs[tb])

            wq_sb = wqop.tile([P, ND // 2, 2, E], fp8, tag="wqo", name="wq")
            nc.sync.dma_start(wq_sb[:], wq8[:])
            qa = [bigA.tile([P, ND // 2, 2, 512], fp8, tag="A",
                            name=f"qa{tb}") for tb in range(TB)]

            # K-proj interleaved with Q-proj h0: K accumulates in psacc,
            # Q0 in the idle psy+pso slots.  DMA emission is per-np so
            # kT8[0] transfers immediately and qa8 arrives np-by-np.
            kaccs = acc4("kacc")
            q0accs = [
                psy.tile([P, 512], f32, tag="y", name="q0acc0"),
                psy.tile([P, 512], f32, tag="y", name="q0acc1"),
                pso.tile([P, 512], f32, tag="o", name="q0acc2"),
                pso.tile([P, 512], f32, tag="o", name="q0acc3"),
            ]
            for np_ in range(ND // 2):
                a8 = actskv.tile([P, 2, T], fp8, tag="akv", name=f"k8{np_}")
                nc.sync.dma_start(a8[:], kT8[np_])
                for tb in range(TB):
                    nc.sync.dma_start(qa[tb][:, np_, :, :],
                                      qT8[tb, :, np_, :, :])
                for tb in range(TB):
                    nc.tensor.matmul(
                        kaccs[tb], wk_sb[:, np_, :, :],
                        a8[:, :, tb * 512:(tb + 1) * 512],
                        start=(np_ == 0), stop=(np_ == ND // 2 - 1),
                        perf_mode=mybir.MatmulPerfMode.DoubleRow)
                    if np_ == ND // 2 - 1:
                        nc.vector.tensor_copy(
                            kt_sb[:, tb * 512:(tb + 1) * 512], kaccs[tb])
                for tb in range(TB):
                    nc.tensor.matmul(
                        q0accs[tb][:],
                        wq_sb[:, np_, :, 0:P],
                        qa[tb][:, np_, :, :],
                        start=(np_ == 0), stop=(np_ == ND // 2 - 1),
                        perf_mode=mybir.MatmulPerfMode.DoubleRow)
                    if np_ == ND // 2 - 1:
                        nc.vector.tensor_copy(
                            qt_sb[:, tb * 512:(tb + 1) * 512],
                            q0accs[tb][:])
            l2norm_scales(kt_sb[:], HPG, None)
            l2norm_scales(qt_sb[:, 0:T], 0, gs_sb[:, 0:1])

            def proj_q(h):
                accs = acc4(f"qacc{h}_")
                for np_ in range(ND // 2):
                    for tb in range(TB):
                        nc.tensor.matmul(
                            accs[tb],
                            wq_sb[:, np_, :, h * P:(h + 1) * P],
                            qa[tb][:, np_, :, :],
                            start=(np_ == 0), stop=(np_ == ND // 2 - 1),
                            perf_mode=mybir.MatmulPerfMode.DoubleRow)
                for tb in range(TB):
                    nc.vector.tensor_copy(
                        qt_sb[:, h * T + tb * 512:h * T + (tb + 1) * 512],
                        accs[tb])
                l2norm_scales(qt_sb[:, h * T:(h + 1) * T], h,
                              gs_sb[:, h:h + 1])

            l2norm_apply(kt_sb[:], HPG)

            # V projection interleaved with Q-proj h1: the vT DMA streams
            # behind Q compute; V accumulates in the idle pso pool.
            wv_sb = wkvp.tile([P, ND * DK], bf16, tag="wk", name="wv_sb")
            nc.sync.dma_start(wv_sb[:], wvt[:])
            vt_stage = bigB.tile([P, T], bf16, tag="B", name="vt_stage")
            vaccs = acc4("vacc")
            q1accs = [
                psy.tile([P, 512], f32, tag="y", name="q1acc0"),
                psy.tile([P, 512], f32, tag="y", name="q1acc1"),
                pso.tile([P, 512], f32, tag="o", name="q1acc2"),
                pso.tile([P, 512], f32, tag="o", name="q1acc3"),
            ]
            for np_ in range(ND // 2):
                for jj in range(2):
                    n = 2 * np_ + jj
                    a = actskv.tile([P, T], bf16, tag="akv", name=f"v{n}")
                    nc.sync.dma_start(a[:], vT[n * P:(n + 1) * P, :])
                    for tb in range(TB):
                        nc.tensor.matmul(
                            vaccs[tb],
                            wv_sb[:, n * DK:(n + 1) * DK],
                            a[:, tb * 512:(tb + 1) * 512],
                            start=(n == 0), stop=(n == ND - 1))
                        if n == ND - 1:
                            nc.vector.tensor_copy(
                                vt_stage[:, tb * 512:(tb + 1) * 512],
                                vaccs[tb])
                for tb in range(TB):
                    nc.tensor.matmul(
                        q1accs[tb][:],
                        wq_sb[:, np_, :, 1 * P:2 * P],
                        qa[tb][:, np_, :, :],
                        start=(np_ == 0), stop=(np_ == ND // 2 - 1),
                        perf_mode=mybir.MatmulPerfMode.DoubleRow)
                    if np_ == ND // 2 - 1:
                        nc.vector.tensor_copy(
                            qt_sb[:, 1 * T + tb * 512:
                                  1 * T + (tb + 1) * 512],
                            q1accs[tb][:])
            l2norm_scales(qt_sb[:, 1 * T:2 * T], 1, gs_sb[:, 1:2])
            l2norm_apply(qt_sb[:, 0 * T:1 * T], 0)
            proj_q(2)
            l2norm_apply(qt_sb[:, 1 * T:2 * T], 1)
            proj_q(3)
            l2norm_apply(qt_sb[:, 2 * T:3 * T], 2)

            # Early per-head stage1 for q-blocks 0/1: S^T matmuls for
            # already-applied heads fill the PE while the h3 norm chain
            # drains; h3's stage1 follows as soon as its apply lands.
            s0, s1, s2, s3 = {}, {}, {}, {}
            stage1(0, [0], s0)
            stage1(1, [0], s1)
            stage1(0, [1], s0)
            stage1(1, [1], s1)
            l2norm_apply(qt_sb[:, 3 * T:4 * T], 3)
            stage1(0, [2], s0)
            stage1(1, [2], s1)
            stage1(0, [3], s0)
            stage1(1, [3], s1)
            # V transposes: PE filler right before attention needs vtm.
            for n in range(NT):
                tp = psy.tile([P, P], bf16, tag="y", name=f"tp{n}")
                nc.tensor.transpose(
                    tp[:], vt_stage[:, n * P:(n + 1) * P], identB[:])
                nc.vector.tensor_copy(vtm_sb[:, n * P:(n + 1) * P], tp[:])

            wo_sb = wqop.tile([P, HPG * D], bf16, tag="wqo", name="wo")
            nc.sync.dma_start(wo_sb[:], wot[:])

            # ------------- phase B: attention + out projection ----------
            ytq0 = stage2(0, s0)
            stage1(2, range(HPG), s2)
            oproj(0, ytq0)
            ytq1 = stage2(1, s1)
            stage1(3, range(HPG), s3)
            oproj(1, ytq1)
            ytq2 = stage2(2, s2)
            ytq3 = stage2(3, s3, early_recip=True)
            oproj(2, ytq2)
            oproj(3, ytq3)

    nc.compile()
    return nc


def make_in_maps(q, k, v, Wq, Wk, Wv, Wo, g):
    import ml_dtypes
    st = ml_dtypes.bfloat16
    f8 = ml_dtypes.float8_e4m3
    in_maps = []
    act_t = {}
    for b in range(B):
        qTb = q[b].T  # (D, T)
        # qT8[tb, p, np, j, c] = qT[(2np+j)*128+p, tb*512+c]
        qT8 = np.ascontiguousarray(
            qTb.reshape(ND // 2, 2, P, TB, 512).transpose(3, 2, 0, 1, 4)
        ).astype(f8)
        kT8 = np.ascontiguousarray(
            k[b].T.reshape(ND // 2, 2, P, T).transpose(0, 2, 1, 3)
        ).astype(f8)
        act_t[b] = (
            qT8,
            kT8,
            np.ascontiguousarray(v[b].T).astype(st),
        )

    def wtile(wT, cols):  # wT: (D, cols) -> [P, ND*cols] row-tiled
        return np.ascontiguousarray(
            np.ascontiguousarray(wT).reshape(-1, P, cols)
            .transpose(1, 0, 2).reshape(P, -1)).astype(st)

    def w8tile(wT, cols):  # wT: (D, cols) -> [P, ND//2, 2, cols] fp8 x32
        return np.ascontiguousarray(
            (np.asarray(wT) * 32.0).reshape(ND // 2, 2, P, cols)
            .transpose(2, 0, 1, 3)).astype(f8)

    g_flat = np.asarray(g, dtype=np.float32).reshape(H)
    for c in range(8):
        b, gi = divmod(c, KVH)
        qT8, kT8b, vTb = act_t[b]
        e0 = gi * E
        gvals = g_flat[gi * HPG:(gi + 1) * HPG] / math.sqrt(DK)
        in_maps.append({
            "qT8": qT8, "kT8": kT8b, "vT": vTb,
            "wq8": w8tile(Wq[e0:e0 + E, :].T, E),
            "wk8": w8tile(Wk[gi * DK:(gi + 1) * DK, :].T, DK),
            "wvt": wtile(Wv[gi * DK:(gi + 1) * DK, :].T, DK),
            "wot": wtile(Wo[:, e0:e0 + E].T, D),
            "gs16": np.broadcast_to(gvals[None, :], (NT, HPG)).copy(),
        })
    return in_maps


_cached = {}


def kernel(q, k, v, Wq, Wk, Wv, Wo, g, _trace=False, _tmpdir=None):
    if "nc" not in _cached:
        _cached["nc"] = build_kernel()
    nc = _cached["nc"]
    in_maps = make_in_maps(
        np.asarray(q, np.float32), np.asarray(k, np.float32),
        np.asarray(v, np.float32), np.asarray(Wq, np.float32),
        np.asarray(Wk, np.float32), np.asarray(Wv, np.float32),
        np.asarray(Wo, np.float32), g)
    res = run_bass_kernel_spmd(
        nc, in_maps, list(range(8)), trace=_trace, tmpdir=_tmpdir)
    out = np.empty((B, T, D), dtype=np.float32)
    for b in range(B):
        acc = res.results[4 * b]["outT"].astype(np.float32)
        for gi in range(1, KVH):
            acc += res.results[4 * b + gi]["outT"].astype(np.float32)
        out[b] = acc.T
    kernel.last_results = res
    return out

